# revision 1
# baseline (speedup 1.0000x reference)
import sys

sys.path.insert(0, '/opt/trn_rl_repo')

import ml_dtypes
import numpy as np
import concourse.bass as bass
import concourse.mybir as mybir
import concourse.tile as tile
from concourse import bacc, bass_utils

F32 = mybir.dt.float32
F32R = mybir.dt.float32r
BF16 = mybir.dt.bfloat16
FP16 = mybir.dt.float16
AF = mybir.ActivationFunctionType

D_MODEL = 1024
N_HEADS = 16
D_HEAD = 64
SEQ = 2048
BATCH = 2
N_CORES = 8
HPC = 4             # heads per core
CSL = HPC * D_HEAD  # 256: qkv feature slice per core
NT_D = D_MODEL // 128  # 8
NQ = SEQ // 512     # 4 q-chunks
GROUPS = [[0, 1, 2, 3], [4, 5, 6, 7]]

_cache = {}


def _build():
    nc = bacc.Bacc("TRN2", target_bir_lowering=False, debug=False,
                   num_devices=N_CORES)
    xt_in = nc.dram_tensor("xt", [D_MODEL, SEQ], FP16, kind="ExternalInput").ap()
    wq_in = nc.dram_tensor("wq", [128, NT_D * CSL], FP16, kind="ExternalInput").ap()
    wk_in = nc.dram_tensor("wk", [128, NT_D * CSL], FP16, kind="ExternalInput").ap()
    wv_in = nc.dram_tensor("wv", [128, NT_D * CSL], FP16, kind="ExternalInput").ap()
    wo_in = nc.dram_tensor("wo", [128, 2 * D_MODEL], FP16, kind="ExternalInput").ap()
    bqk_in = nc.dram_tensor("bqk", [128, 4], F32, kind="ExternalInput").ap()
    bv_in = nc.dram_tensor("bv", [128, CSL], F32, kind="ExternalInput").ap()
    bo4_in = nc.dram_tensor("bo4", [128, D_MODEL], F32, kind="ExternalInput").ap()
    nm_in = nc.dram_tensor("negm16", [128, 256], FP16, kind="ExternalInput").ap()
    id_in = nc.dram_tensor("id16", [128, 128], FP16, kind="ExternalInput").ap()
    o16_in = nc.dram_tensor("ones16", [1, 128], FP16, kind="ExternalInput").ap()
    bo4r_in = nc.dram_tensor("bo4r", [1, D_MODEL], FP16, kind="ExternalInput").ap()
    out = nc.dram_tensor("out", [NQ * 128, D_MODEL], FP16,
                         kind="ExternalOutput").ap()

    with tile.TileContext(nc) as tc:
        _body(nc, tc, xt_in, wq_in, wk_in, wv_in, wo_in, bqk_in, bv_in,
              bo4_in, nm_in, id_in, o16_in, bo4r_in, out)
    nc.compile()
    return nc


def _body(nc, tc, xt_in, wq_in, wk_in, wv_in, wo_in, bqk_in, bv_in,
          bo4_in, nm_in, id_in, o16_in, bo4r_in, out):
    from contextlib import ExitStack
    ctx = ExitStack()
    with ctx:
        const = ctx.enter_context(tc.tile_pool(name="const", bufs=1))
        wpool = ctx.enter_context(tc.tile_pool(name="wpool", bufs=1))
        xtpool = ctx.enter_context(tc.tile_pool(name="xtpool", bufs=1))
        qkpool = ctx.enter_context(tc.tile_pool(name="qkpool", bufs=1))
        vpool = ctx.enter_context(tc.tile_pool(name="vpool", bufs=1))
        htpool = ctx.enter_context(tc.tile_pool(name="htpool", bufs=1))
        exp_pool = ctx.enter_context(tc.tile_pool(name="exp_pool", bufs=5))
        misc_c = ctx.enter_context(tc.tile_pool(name="misc_c", bufs=2))
        stpool = ctx.enter_context(tc.tile_pool(name="stpool", bufs=2))
        drpool = ctx.enter_context(tc.tile_pool(name="drpool", bufs=1, space="DRAM"))
        ps_pair = ctx.enter_context(tc.tile_pool(name="ps_pair", bufs=2, space="PSUM"))
        ps_att = ctx.enter_context(tc.tile_pool(name="ps_att", bufs=1, space="PSUM"))
        ps_m = ctx.enter_context(tc.tile_pool(name="ps_m", bufs=2, space="PSUM"))

        # ---- startup order: wq head, chunk-0 x columns, wk, consts, rest ----
        wq_t = wpool.tile([128, NT_D * CSL], FP16, name="w_wq")
        nc.scalar.dma_start(wq_t[:, 0:CSL], wq_in[:, 0:CSL])
        xT = [xtpool.tile([128, SEQ], FP16, name=f"xT{dj}", tag=f"xT{dj}")
              for dj in range(NT_D)]
        nc.sync.dma_start(xT[0][:, 0:512], xt_in[0:128, 0:512])
        nc.scalar.dma_start(wq_t[:, CSL:NT_D * CSL], wq_in[:, CSL:NT_D * CSL])
        for dj in range(1, NT_D):
            nc.sync.dma_start(xT[dj][:, 0:512],
                              xt_in[128 * dj:128 * (dj + 1), 0:512])
        wk_t = wpool.tile([128, NT_D * CSL], FP16, name="w_wk")
        nc.scalar.dma_start(wk_t[:], wk_in[:])
        bqk = const.tile([128, 4], F32)
        nc.scalar.dma_start(bqk[:], bqk_in[:])
        wv_t = wpool.tile([128, NT_D * CSL], FP16, name="w_wv")
        nc.scalar.dma_start(wv_t[:], wv_in[:])
        bv2 = const.tile([128, CSL], F32)
        nc.scalar.dma_start(bv2[:], bv_in[:])
        ones16 = const.tile([1, 128], FP16)
        nc.scalar.dma_start(ones16[:], o16_in[:])
        negm16 = const.tile([128, 256], FP16)
        nc.scalar.dma_start(negm16[:], nm_in[:])
        id16 = const.tile([128, 128], FP16)
        nc.scalar.dma_start(id16[:], id_in[:])
        wo_t = wpool.tile([128, 2 * D_MODEL], FP16, name="w_wo")
        nc.scalar.dma_start(wo_t[:], wo_in[:])
        bo4 = const.tile([128, D_MODEL], F32)
        nc.scalar.dma_start(bo4[:], bo4_in[:])
        bo4r = const.tile([1, D_MODEL], FP16)
        nc.scalar.dma_start(bo4r[:], bo4r_in[:])

        # ---- rest of x^T columns ----
        for dj in range(NT_D):
            nc.sync.dma_start(xT[dj][:, 512:SEQ],
                              xt_in[128 * dj:128 * (dj + 1), 512:SEQ])

        # ---- persistent activations ----
        qt, kt, hT = [], [], []
        for ci in range(2):
            qt.append(qkpool.tile([128, SEQ], FP16, name=f"qt{ci}", tag=f"qt{ci}"))
            kt.append(qkpool.tile([128, SEQ], FP16, name=f"kt{ci}", tag=f"kt{ci}"))
            hT.append(htpool.tile([128, SEQ], FP16, name=f"hT{ci}", tag=f"hT{ci}"))
        vt = [vpool.tile([128, HPC * 65], FP16, name=f"vt{si}", tag=f"vt{si}")
              for si in range(16)]

        def emit_qk_half(sj, ci, w_t, bcol, dst, half, state):
            """Half a projection slice: 4 matmuls (+ bias-add on 2nd half)."""
            if half == 0:
                state["pp"] = ps_m.tile([128, 512], F32, name="pp", tag="m")
            pp = state["pp"]
            for dj in range(4 * half, 4 * half + 4):
                nc.tensor.matmul(
                    pp[:],
                    w_t[:, dj * CSL + 128 * ci:dj * CSL + 128 * (ci + 1)],
                    xT[dj][:, 512 * sj:512 * (sj + 1)],
                    start=(dj == 0), stop=(dj == NT_D - 1),
                    skip_group_check=True)
            if half == 1:
                nc.vector.tensor_scalar_add(
                    dst[ci][:, 512 * sj:512 * (sj + 1)], pp[:],
                    bqk[:, bcol + ci:bcol + ci + 1])

        def emit_qk(sj, ci, w_t, bcol, dst):
            state = {}
            emit_qk_half(sj, ci, w_t, bcol, dst, 0, state)
            emit_qk_half(sj, ci, w_t, bcol, dst, 1, state)

        def emit_v_half(sj, sl, half, state):
            """Half a v s-tile: 4 matmuls (+ ones memset and add on 2nd)."""
            si = 4 * sj + sl
            if half == 0:
                state["pv"] = ps_m.tile([128, 512], F32, name="pv", tag="m")
            pv = state["pv"]
            for dj in range(4 * half, 4 * half + 4):
                nc.tensor.matmul(
                    pv[:, 0:CSL],
                    xT[dj][:, 128 * si:128 * (si + 1)],
                    wv_t[:, dj * CSL:(dj + 1) * CSL],
                    start=(dj == 0), stop=(dj == NT_D - 1),
                    skip_group_check=True)
            if half == 1:
                nc.vector.memset(
                    vt[si].rearrange("p (h e) -> p h e", e=65)[:, :, 64:65], 1.0)
                nc.vector.tensor_add(
                    vt[si].rearrange("p (h e) -> p h e", e=65)[:, :, 0:64],
                    pv[:, 0:CSL].rearrange("p (h e) -> p h e", e=64),
                    bv2.rearrange("p (h e) -> p h e", e=64))

        def emit_v(sj, sl):
            state = {}
            emit_v_half(sj, sl, 0, state)
            emit_v_half(sj, sl, 1, state)

        pending = []

        def pop_pending(n):
            for _ in range(min(n, len(pending))):
                pending.pop(0)()

        deferred_norm = [None]

        def emit_norm(sj, ci, pa):
            # softmax denominators -> reciprocal -> broadcast -> scale
            rt = misc_c.tile([1, 1024], FP16, name="rt", tag="rt")
            with nc.allow_low_precision(reason="softmax 1/denom in fp16"):
                nc.vector.reciprocal(rt[:, 0:512], pa[0][64:65, :])
                nc.vector.reciprocal(rt[:, 512:1024], pa[1][64:65, :])
            pop_pending(2)  # cover the reciprocal wait with filler matmuls
            pb = ps_m.tile([128, 512], F32, name="pb", tag="m")
            nc.tensor.matmul(pb[0:64, :], ones16[0:1, 0:64], rt[:, 0:512],
                             start=True, stop=True)
            nc.tensor.matmul(pb[64:128, :], ones16[0:1, 0:64], rt[:, 512:1024],
                             start=True, stop=True, tile_position=(0, 64))
            bc = misc_c.tile([128, 512], FP16, name="bc", tag="bc")
            nc.scalar.copy(bc[:], pb[:])
            for hh in range(2):
                nc.vector.tensor_mul(
                    hT[ci][64 * hh:64 * (hh + 1), 512 * sj:512 * (sj + 1)],
                    pa[hh][0:64, :], bc[64 * hh:64 * (hh + 1), :])

        def flush_norm():
            if deferred_norm[0] is not None:
                emit_norm(*deferred_norm[0])
                deferred_norm[0] = None

        def emit_attention(sj):
            nk = 4 * sj + 4
            # narrow diag tile first (fast exp refill), narrow-ish last
            order = list(range(4 * sj + 3, 4 * sj - 1, -1)) + \
                    list(range(4 * sj))
            for ci in range(2):
                pa = [ps_att.tile([65, 512], F32, name=f"pa{hh}", tag=f"att{hh}")
                      for hh in range(2)]
                for pos, ki in enumerate(order):
                    r = ki - 4 * sj
                    c0 = 0 if r < 0 else 128 * r
                    ps = ps_pair.tile([128, 1024], F32, name="ps", tag="sp")
                    for hh in range(2):
                        p0 = 64 * hh
                        nc.tensor.matmul(
                            ps[:, 512 * hh + c0:512 * (hh + 1)],
                            kt[ci][p0:p0 + 64, 128 * ki:128 * (ki + 1)],
                            qt[ci][p0:p0 + 64, 512 * sj + c0:512 * (sj + 1)],
                            start=True, stop=(r < 0))
                    if r >= 0:
                        # causal mask via PE, both heads: ps += I^T @ [mask|mask]
                        nc.tensor.matmul(
                            ps.rearrange("p (h c) -> p h c", h=2)[:, :, c0:c0 + 128],
                            id16[:], negm16[:],
                            start=False, stop=True, skip_group_check=True)
                    et = exp_pool.tile([128, 1024], FP16, name="et", tag="et")
                    nc.scalar.activation(
                        et.rearrange("p (h c) -> p h c", h=2)[:, :, c0:512],
                        ps.rearrange("p (h c) -> p h c", h=2)[:, :, c0:512],
                        AF.Exp)
                    for hh in range(2):
                        h_local = 2 * ci + hh
                        nc.tensor.matmul(
                            pa[hh][:, c0:512],
                            vt[ki][:, 65 * h_local:65 * h_local + 65],
                            et[:, 512 * hh + c0:512 * (hh + 1)],
                            start=(pos == 0), stop=(pos == nk - 1),
                            skip_group_check=True)
                    if pos % 2 == 1:
                        pop_pending(1)
                emit_norm(sj, ci, pa)
                pop_pending(1)

        wo_stage = {}

        def emit_wo_half(sj, sl, h, tail):
            """One 128-row x 512-col block of partial out = hT@Wo + bo/4."""
            if sl == 0 and h == 0:
                wo_stage[sj] = (
                    stpool.tile([128, 4 * D_MODEL], FP16, name=f"pst{sj}",
                                tag="pst"),
                    drpool.tile([512, D_MODEL], FP16, name=f"prs{sj}",
                                tag=f"prs{sj}"))
            pstage, prs = wo_stage[sj]
            s0 = 512 * sj + 128 * sl
            on_act = tail and h == 1
            po = ps_m.tile([128, 512], F32, name="po", tag="m")
            for ci in range(2):
                nc.tensor.matmul(
                    po[:],
                    hT[ci][:, s0:s0 + 128],
                    wo_t[:, ci * D_MODEL + 512 * h:ci * D_MODEL + 512 * (h + 1)],
                    start=(ci == 0), stop=(ci == 1 and not on_act))
            dst = pstage[:, D_MODEL * sl + 512 * h:D_MODEL * sl + 512 * (h + 1)]
            if on_act:
                nc.tensor.matmul(po[:], ones16[:],
                                 bo4r[:, 512 * h:512 * (h + 1)],
                                 start=False, stop=True)
                nc.scalar.copy(dst, po[:])
            else:
                nc.vector.tensor_add(dst, po[:], bo4[:, 512 * h:512 * (h + 1)])
            if h == 1:
                # stage this row-slice to DRAM
                nc.sync.dma_start(
                    prs.rearrange("(s p) c -> p s c", p=128)[:, sl:sl + 1, :],
                    pstage.rearrange("p (s c) -> p s c", c=D_MODEL)[:, sl:sl + 1, :])
                if sl == 3:
                    pro = drpool.tile([128, D_MODEL], FP16, name=f"pro{sj}",
                                      tag=f"pro{sj}")
                    nc.gpsimd.collective_compute(
                        "ReduceScatter", mybir.AluOpType.add,
                        replica_groups=GROUPS, ins=[prs[:]], outs=[pro[:]])
                    nc.sync.dma_start(out[128 * sj:128 * (sj + 1), :], pro[:])

        def emit_wo_sl(sj, sl, tail):
            emit_wo_half(sj, sl, 0, tail)
            emit_wo_half(sj, sl, 1, tail)

        # ---- pipeline over q-chunks ----
        # chunk-0 q/k run dj-major so PE consumes x slices as they arrive
        qk0 = [(wq_t, 0, qt, 0), (wk_t, 2, kt, 0),
               (wq_t, 0, qt, 1), (wk_t, 2, kt, 1)]
        pps0 = []
        for i in range(2):
            t = ps_pair.tile([128, 1024], F32, name=f"pp0_{i}", tag="sp")
            pps0 += [t[:, 0:512], t[:, 512:1024]]
        for dj in range(NT_D):
            for u, (w_t, bcol, dst, ci) in enumerate(qk0):
                nc.tensor.matmul(
                    pps0[u],
                    w_t[:, dj * CSL + 128 * ci:dj * CSL + 128 * (ci + 1)],
                    xT[dj][:, 0:512],
                    start=(dj == 0), stop=(dj == NT_D - 1),
                    skip_group_check=True)
        for u, (w_t, bcol, dst, ci) in enumerate(qk0):
            nc.vector.tensor_scalar_add(dst[ci][:, 0:512], pps0[u],
                                        bqk[:, bcol + ci:bcol + ci + 1])
        for sl in range(4):
            emit_v(0, sl)
        for sj in range(NQ):
            if sj > 0:
                for u, (sl, h) in enumerate([(l, hh) for l in range(4)
                                             for hh in range(2)]):
                    pending.insert(u, lambda s=sj - 1, l=sl, hh=h:
                                   emit_wo_half(s, l, hh, False))
            if sj + 1 < NQ:
                nsj = sj + 1
                for ci in range(2):
                    for w_t, bcol, dst in [(wq_t, 0, qt), (wk_t, 2, kt)]:
                        st = {}
                        for half in range(2):
                            pending.append(
                                lambda c=ci, s=nsj, w=w_t, b=bcol, d=dst,
                                h=half, t=st: emit_qk_half(s, c, w, b, d, h, t))
                for sl in range(4):
                    st = {}
                    for half in range(2):
                        pending.append(
                            lambda s=nsj, l=sl, h=half, t=st:
                            emit_v_half(s, l, h, t))
            emit_attention(sj)
            pop_pending(len(pending))
        for sl in range(4):
            emit_wo_sl(NQ - 1, sl, True)


def _consts():
    kk = np.arange(128)[:, None]
    qq = np.arange(128)[None, :]
    negm16 = np.where(kk <= qq, 0.0, -60000.0).astype(np.float16)
    negm16 = np.concatenate([negm16, negm16], axis=1)
    id16 = np.eye(128, dtype=np.float16)
    return negm16, id16


def _pack_w(w):
    """[1024, C] -> [128, 8*C] fp16 with d-tile t at column block t."""
    c = w.shape[1]
    return np.ascontiguousarray(
        w.reshape(NT_D, 128, c).transpose(1, 0, 2).reshape(128, NT_D * c)
    ).astype(np.float16)


def kernel(x, Wq, bq, Wk, bk, Wv, bv, Wo, bo):
    x = np.asarray(x, dtype=np.float32)
    Wq = np.asarray(Wq, dtype=np.float32)
    bq = np.asarray(bq, dtype=np.float32)
    Wk = np.asarray(Wk, dtype=np.float32)
    bk = np.asarray(bk, dtype=np.float32)
    Wv = np.asarray(Wv, dtype=np.float32)
    bv = np.asarray(bv, dtype=np.float32)
    Wo = np.asarray(Wo, dtype=np.float32)
    bo = np.asarray(bo, dtype=np.float32)

    if "nc" not in _cache:
        _cache["nc"] = _build()
    nc = _cache["nc"]

    negm16, id16 = _consts()
    scale = 1.0 / np.sqrt(np.float32(D_HEAD))
    ones16 = np.ones((1, 128), dtype=np.float16)
    bo4 = np.ascontiguousarray(
        np.broadcast_to((bo / 4.0)[None, :], (128, D_MODEL))).astype(np.float32)
    bo4r = (bo / 4.0)[None, :].astype(np.float16)
    in_maps = []
    for core in range(N_CORES):
        b, g = divmod(core, HPC)
        csl = slice(CSL * g, CSL * (g + 1))
        wo_loc = Wo[csl, :]  # [256, 1024]
        wo_pack = np.ascontiguousarray(
            wo_loc.reshape(2, 128, D_MODEL).transpose(1, 0, 2).reshape(
                128, 2 * D_MODEL)).astype(np.float16)
        bqk = np.stack([
            bq[csl][0:128] * scale, bq[csl][128:256] * scale,
            bk[csl][0:128], bk[csl][128:256],
        ], axis=1).astype(np.float32)
        in_maps.append({
            "xt": np.ascontiguousarray(x[b].T).astype(np.float16),
            "wq": _pack_w(Wq[:, csl] * scale),
            "wk": _pack_w(Wk[:, csl]),
            "wv": _pack_w(Wv[:, csl]),
            "wo": wo_pack,
            "bqk": np.ascontiguousarray(bqk),
            "bv": np.ascontiguousarray(
                np.broadcast_to(bv[None, csl], (128, CSL))).astype(np.float32),
            "bo4": bo4,
            "negm16": negm16,
            "id16": id16,
            "ones16": ones16,
            "bo4r": bo4r,
        })

    # the axon terminal occasionally reports a transient
    # NRT_EXEC_UNIT_UNRECOVERABLE; retries with backoff recover it
    import time as _time
    for attempt in range(3):
        try:
            res = bass_utils.run_bass_kernel_spmd(
                nc, in_maps, core_ids=list(range(N_CORES)))
            break
        except Exception:
            if attempt == 2:
                raise
            _time.sleep(5.0 * (attempt + 1))

    full = np.empty((BATCH, SEQ, D_MODEL), dtype=np.float32)
    for core in range(N_CORES):
        b, g = divmod(core, HPC)
        o = np.asarray(res.results[core]["out"]).astype(np.float32)
        for qi in range(NQ):
            full[b, 512 * qi + 128 * g:512 * qi + 128 * (g + 1), :] = \
                o[128 * qi:128 * (qi + 1), :]
    return full



# revision 33
# speedup vs baseline: 1.2043x; 1.2043x over previous
import sys

sys.path.insert(0, '/opt/trn_rl_repo')

import ml_dtypes
import numpy as np
import concourse.bass as bass
import concourse.mybir as mybir
import concourse.tile as tile
from concourse import bacc, bass_utils

F32 = mybir.dt.float32
BF16 = mybir.dt.bfloat16
FP16 = mybir.dt.float16
FP8 = mybir.dt.float8e4
AF = mybir.ActivationFunctionType
DR = mybir.MatmulPerfMode.DoubleRow
MUL = mybir.AluOpType.mult
ADD = mybir.AluOpType.add

D_MODEL = 1024
N_HEADS = 16
D_HEAD = 64
SEQ = 2048
BATCH = 2
N_CORES = 8
HPC = 4             # heads per core
CSL = HPC * D_HEAD  # 256: qkv feature slice per core
NT_D = D_MODEL // 128  # 8
NQ = SEQ // 512     # 4 q-chunks
GROUPS = [[0, 1, 2, 3], [4, 5, 6, 7]]
WS = 32.0           # fp8 weight pre-scale (host side)

_cache = {}


def _build():
    nc = bacc.Bacc("TRN2", target_bir_lowering=False, debug=False,
                   num_devices=N_CORES)
    xt_in = nc.dram_tensor("xt", [D_MODEL, SEQ], FP8, kind="ExternalInput").ap()
    w_in = nc.dram_tensor("wqkv", [128, 4 * NT_D * CSL], FP8,
                          kind="ExternalInput").ap()
    c32_in = nc.dram_tensor("cst32", [128, 4 + 2 * CSL + NT_D], F32,
                            kind="ExternalInput").ap()
    c16_in = nc.dram_tensor("cst16", [128, 384 + 2 * NT_D * CSL + 2048], FP16,
                            kind="ExternalInput").ap()
    out = nc.dram_tensor("out", [NQ * 256, 512], FP16,
                         kind="ExternalOutput").ap()

    with tile.TileContext(nc) as tc:
        _body(nc, tc, xt_in, w_in, c32_in, c16_in, out)
    nc.compile()
    return nc


def _body(nc, tc, xt_in, w_in, c32_in, c16_in, out):
    from contextlib import ExitStack
    ctx = ExitStack()
    with ctx:
        const = ctx.enter_context(tc.tile_pool(name="const", bufs=1))
        wpool = ctx.enter_context(tc.tile_pool(name="wpool", bufs=1))
        xtpool = ctx.enter_context(tc.tile_pool(name="xtpool", bufs=1))
        qkpool = ctx.enter_context(tc.tile_pool(name="qkpool", bufs=1))
        vpool = ctx.enter_context(tc.tile_pool(name="vpool", bufs=1))
        htpool = ctx.enter_context(tc.tile_pool(name="htpool", bufs=1))
        exp_pool = ctx.enter_context(tc.tile_pool(name="exp_pool", bufs=6))
        misc_c = ctx.enter_context(tc.tile_pool(name="misc_c", bufs=2))
        stpool = ctx.enter_context(tc.tile_pool(name="stpool", bufs=2))
        drpool = ctx.enter_context(tc.tile_pool(name="drpool", bufs=1, space="DRAM"))
        ps_pair = ctx.enter_context(tc.tile_pool(name="ps_pair", bufs=2, space="PSUM"))
        ps_att = ctx.enter_context(tc.tile_pool(name="ps_att", bufs=1, space="PSUM"))
        ps_m = ctx.enter_context(tc.tile_pool(name="ps_m", bufs=2, space="PSUM"))

        # ---- startup: few fat DMAs (HWDGE desc-gen is ~630ns each) ----
        xT = xtpool.tile([128, NT_D * SEQ], FP8, name="xT", tag="xT")
        xTv = xT.rearrange("p (d s) -> p d s", s=SEQ)
        xdr = xt_in.rearrange("(d p) s -> p d s", p=128)
        nc.sync.dma_start(xTv[:, :, 0:512], xdr[:, :, 0:512])
        wt = wpool.tile([128, 4 * NT_D * CSL], FP8, name="w_all")
        nc.scalar.dma_start(wt[:, 0:2 * NT_D * CSL], w_in[:, 0:2 * NT_D * CSL])
        c16 = const.tile([128, 384 + 2 * NT_D * CSL + 2048], FP16)
        nc.scalar.dma_start(c16[:, 0:384], c16_in[:, 0:384])
        c32 = const.tile([128, 4 + 2 * CSL + NT_D], F32)
        nc.scalar.dma_start(c32[:], c32_in[:])
        nc.scalar.dma_start(wt[:, 2 * NT_D * CSL:], w_in[:, 2 * NT_D * CSL:])
        nc.sync.dma_start(xTv[:, :, 512:SEQ], xdr[:, :, 512:SEQ])
        nc.scalar.dma_start(c16[:, 384:], c16_in[:, 384:])
        wq_t = wt[:, 0:NT_D * CSL]
        wk_t = wt[:, NT_D * CSL:2 * NT_D * CSL]
        wv_t = wt[:, 2 * NT_D * CSL:3 * NT_D * CSL]
        wo_t = wt[:, 3 * NT_D * CSL:4 * NT_D * CSL]
        bqk = c32[:, 0:4]
        bv32 = c32[:, 4:4 + CSL]
        boT4 = c32[:, 4 + CSL:4 + CSL + NT_D]
        negm16 = c16[:, 0:256]
        id16 = c16[:, 256:384]
        wv16 = c16[:, 384:384 + NT_D * CSL]
        wv16v = wv16.rearrange("p (d m) -> p d m", m=CSL)
        wo16v = c16[:, 384 + NT_D * CSL:384 + 2 * NT_D * CSL].rearrange(
            "p (s c) -> p s c", c=D_MODEL)
        xT16v = c16[:, 384 + 2 * NT_D * CSL:].rearrange(
            "p (d s) -> p d s", s=128)
        bv16 = c32[:, 4 + CSL + NT_D:4 + 2 * CSL + NT_D]
        ones16 = const.tile([1, 128], FP16)
        nc.vector.memset(ones16[:], 1.0)
        actwarm = const.tile([1, 4], FP16)
        nc.vector.memset(actwarm[:], 0.0)
        nc.scalar.activation(actwarm[:], actwarm[:], AF.Exp)
        onesw = const.tile([1, 512], FP16)
        nc.vector.memset(onesw[:], 1.0)
        bo16 = c16[0:1, 384 + 2 * NT_D * CSL + 1024:]

        # ---- persistent activations ----
        qt, kt = [], []
        for ci in range(2):
            qt.append(qkpool.tile([128, SEQ], FP16, name=f"qt{ci}", tag=f"qt{ci}"))
            kt.append(qkpool.tile([128, SEQ], FP16, name=f"kt{ci}", tag=f"kt{ci}"))
        hT = htpool.tile([128, 2 * SEQ], FP8, name="hT", tag="hT")
        hTv = hT.rearrange("p (s c) -> p s c", c=SEQ)
        vtp = [vpool.tile([128, 2 * HPC * 68], FP8, name=f"vt{j}", tag=f"vt{j}")
               for j in range(NT_D)]
        wav = wt.rearrange("p (w d m) -> p w d m", w=4, m=CSL)
        wqv = wav[:, 0]
        wkv = wav[:, 1]
        wvv = wav[:, 2]
        wov = wt.rearrange("p (w s c) -> p w s c", w=4, c=D_MODEL)[:, 3]

        def emit_qk_half(sj, ci, wv_, bcol, dst, half, state):
            """Half a q/k projection slice: 2 DR matmuls (+ bias on 2nd)."""
            if half == 0:
                state["pp"] = ps_m.tile([128, 512], F32, name="pp", tag="m")
            pp = state["pp"]
            for dj in (4 * half, 4 * half + 2):
                nc.tensor.matmul(
                    pp[:],
                    wv_[:, dj:dj + 2, 128 * ci:128 * (ci + 1)],
                    xTv[:, dj:dj + 2, 512 * sj:512 * (sj + 1)],
                    start=(dj == 0), stop=(dj == 6),
                    perf_mode=DR, skip_group_check=True)
            if half == 1:
                nc.vector.tensor_scalar(
                    dst[ci][:, 512 * sj:512 * (sj + 1)], pp[:],
                    1.0 / WS, bqk[:, bcol + ci:bcol + ci + 1],
                    op0=MUL, op1=ADD)

        def emit_qk(sj, ci, wv_, bcol, dst):
            state = {}
            emit_qk_half(sj, ci, wv_, bcol, dst, 0, state)
            emit_qk_half(sj, ci, wv_, bcol, dst, 1, state)

        def emit_v_half(sj, sl, half, state):
            """Half a v s-tile: 2 DR matmuls (+ ones memset and add on 2nd)."""
            si = 4 * sj + sl
            if half == 0:
                state["pv"] = ps_m.tile([128, 512], F32, name="pv", tag="m")
            pv = state["pv"]
            for dj in (4 * half, 4 * half + 2):
                nc.tensor.matmul(
                    pv[:, 0:CSL],
                    xTv[:, dj:dj + 2, 128 * si:128 * (si + 1)],
                    wvv[:, dj:dj + 2, :],
                    start=(dj == 0), stop=(dj == 6),
                    perf_mode=DR, skip_group_check=True)
            if half == 1:
                j, slot = divmod(si, 2)
                vv = vtp[j].rearrange("p (s h e) -> p s h e", s=2, e=68)
                nc.vector.memset(vv[:, slot, :, 64:65], 1.0)
                nc.vector.tensor_add(
                    vv[:, slot, :, 0:64],
                    pv[:, 0:CSL].rearrange("p (h e) -> p h e", e=64),
                    bv32.rearrange("p (h e) -> p h e", e=64))

        def emit_v(sj, sl):
            state = {}
            emit_v_half(sj, sl, 0, state)
            emit_v_half(sj, sl, 1, state)

        pending = []
        pending_wo = []

        def pop_pending(n):
            for _ in range(min(n, len(pending))):
                pending.pop(0)()

        def pop_wo(n=100):
            for _ in range(min(n, len(pending_wo))):
                pending_wo.pop(0)()

        deferred = [None]

        def flush_norm():
            if deferred[0] is not None:
                f = deferred[0]
                deferred[0] = None
                f()

        def emit_norm(sj, ci, pa):
            # softmax denominators -> reciprocal -> broadcast -> scale
            rt = misc_c.tile([1, 1024], FP16, name="rt", tag="rt")
            with nc.allow_low_precision(reason="softmax 1/denom in fp16"):
                nc.vector.reciprocal(rt[:, 0:512], pa[0][64:65, :])
                nc.vector.reciprocal(rt[:, 512:1024], pa[1][64:65, :])
            pb = ps_m.tile([128, 512], F32, name="pb", tag="m")
            nc.tensor.matmul(pb[0:64, :], ones16[0:1, 0:64], rt[:, 0:512],
                             start=True, stop=True)
            nc.tensor.matmul(pb[64:128, :], ones16[0:1, 0:64], rt[:, 512:1024],
                             start=True, stop=True, tile_position=(0, 64))
            bc = misc_c.tile([128, 512], FP16, name="bc", tag="bc")
            nc.vector.tensor_scalar_add(bc[:], pb[:], 0.0)
            pop_pending(2)  # cover the bc wait with filler matmuls
            for hh in range(2):
                nc.vector.tensor_mul(
                    hTv[64 * hh:64 * (hh + 1), ci, 512 * sj:512 * (sj + 1)],
                    pa[hh][0:64, :], bc[64 * hh:64 * (hh + 1), :])

        def emit_attention(sj):
            npairs = 2 * sj + 2
            # diag pairs first (fast psum refill), then off-diag ascending
            order = [2 * sj + 1, 2 * sj] + list(range(2 * sj))
            for ci in range(2):
                pa = [ps_att.tile([65, 512], F32, name=f"pa{hh}", tag=f"att{hh}")
                      for hh in range(2)]

                def emit_av(j, etv, pos):
                    """attn@V for pair j (software-pipelined one pair behind)."""
                    is_diag_hi = (j == 2 * sj + 1)
                    is_diag_lo = (j == 2 * sj)
                    last = (pos == npairs - 1)
                    for hh in range(2):
                        h_local = 2 * ci + hh
                        vv = vtp[j].rearrange(
                            "p (s h e) -> p s h e", s=2, e=68)[:, :, h_local, 0:65]
                        if is_diag_hi:
                            # tiles r2 (c0=256), r3 (c0=384)
                            nc.tensor.matmul(
                                pa[hh][:, 384:512], vv, etv[:, hh, :, 384:512],
                                start=(pos == 0), stop=False,
                                perf_mode=DR, skip_group_check=True)
                            nc.tensor.matmul(
                                pa[hh][:, 256:384], vv[:, 0, :],
                                etv[:, hh, 0, 256:384],
                                start=False, stop=False, skip_group_check=True)
                        elif is_diag_lo:
                            # tiles r0 (c0=0), r1 (c0=128)
                            nc.tensor.matmul(
                                pa[hh][:, 128:512], vv, etv[:, hh, :, 128:512],
                                start=False, stop=False,
                                perf_mode=DR, skip_group_check=True)
                            nc.tensor.matmul(
                                pa[hh][:, 0:128], vv[:, 0, :],
                                etv[:, hh, 0, 0:128],
                                start=False, stop=last, skip_group_check=True)
                        else:
                            nc.tensor.matmul(
                                pa[hh][:, 0:512], vv, etv[:, hh, :, 0:512],
                                start=False, stop=last,
                                perf_mode=DR, skip_group_check=True)

                prev = None
                for pos, j in enumerate(order):
                    et = exp_pool.tile([128, 2048], FP8, name="et", tag="et")
                    etv = et.rearrange("p (h s c) -> p h s c", h=2, c=512)
                    for slot in range(2):
                        ki = 2 * j + slot
                        r = ki - 4 * sj
                        c0 = 0 if r < 0 else 128 * r
                        ps = ps_pair.tile([128, 1024], F32, name="ps", tag="sp")
                        psv = ps.rearrange("p (h c) -> p h c", h=2)
                        for hh in range(2):
                            p0 = 64 * hh
                            nc.tensor.matmul(
                                ps[:, 512 * hh + c0:512 * (hh + 1)],
                                kt[ci][p0:p0 + 64, 128 * ki:128 * (ki + 1)],
                                qt[ci][p0:p0 + 64, 512 * sj + c0:512 * (sj + 1)],
                                start=True, stop=(r < 0))
                        if r >= 0:
                            # causal mask via PE, both heads
                            nc.tensor.matmul(
                                psv[:, :, c0:c0 + 128],
                                id16[:], negm16[:],
                                start=False, stop=True, skip_group_check=True)
                        nc.scalar.activation(
                            etv[:, :, slot, c0:512],
                            psv[:, :, c0:512],
                            AF.Exp)
                    if pos == 0:
                        flush_norm()
                        pop_wo(2)
                        if sj == 0 and ci == 0:
                            pop_pending(4)
                    if prev is not None:
                        emit_av(*prev)
                        pop_wo(2)
                        pop_pending(1)
                    prev = (j, etv, pos)
                if sj == 0 and ci == 0:
                    pop_pending(8)
                emit_av(*prev)
                deferred[0] = lambda s=sj, c=ci, p=pa: emit_norm(s, c, p)
                pop_pending(1)

        wo_stage = {}
        wo_done = set()
        out_dmas = []

        def emit_wo_t(sj, t):
            """One transposed out block [128 dout, 512 seq] = Wo_t.T @ hT."""
            if sj not in wo_stage:
                wo_stage[sj] = (
                    [stpool.tile([128, 2 * 512], FP16, name=f"pst{sj}_{q}",
                                 tag=f"pst{q}") for q in range(4)],
                    drpool.tile([D_MODEL, 512], FP16, name=f"prs{sj}",
                                tag=f"prs{sj}"))
            pstage, prs = wo_stage[sj]
            pstage = pstage[t // 2]
            po = ps_m.tile([128, 512], F32, name="po", tag="m")
            tail = (sj == NQ - 1)
            nc.tensor.matmul(
                po[:],
                wov[:, :, 128 * t:128 * (t + 1)],
                hTv[:, :, 512 * sj:512 * (sj + 1)],
                start=True, stop=not tail, perf_mode=DR, skip_group_check=True)
            c0w = 128 if sj == 0 else 0
            dst = pstage.rearrange("p (t c) -> p t c", c=512)[:, t % 2, c0w:512]
            if tail:
                # bias via PE; staging split ACT/DVE (ACT idles at the tail)
                nc.tensor.matmul(
                    po[:], bo16[0:1, 128 * t:128 * (t + 1)], onesw[0:1, :],
                    start=False, stop=True, skip_group_check=True)
                if t % 2 == 0:
                    nc.scalar.mul(dst, po[:, c0w:512], 1.0 / (WS * WS))
                else:
                    nc.vector.tensor_scalar(
                        dst, po[:, c0w:512], 1.0 / (WS * WS), None, op0=MUL)
            else:
                nc.vector.tensor_scalar(
                    dst, po[:, c0w:512],
                    1.0 / (WS * WS), boT4[:, t:t + 1], op0=MUL, op1=ADD)
            if t % 2 == 1:
                eng = nc.scalar if sj == NQ - 1 else nc.sync
                eng.dma_start(
                    prs.rearrange("(t p) c -> p t c", p=128)[:, t - 1:t + 1, :],
                    pstage.rearrange("p (t c) -> p t c", c=512)[:, :, :])
            if t == NT_D - 1:
                pro = drpool.tile([256, 512], FP16, name=f"pro{sj}",
                                  tag=f"pro{sj}")
                nc.gpsimd.collective_compute(
                    "ReduceScatter", mybir.AluOpType.add,
                    replica_groups=GROUPS, ins=[prs[:]], outs=[pro[:]])
                out_dmas.append(
                    lambda s=sj, p=pro: nc.sync.dma_start(
                        out[256 * s:256 * (s + 1), :], p[:]))

        def emit_side():
            """fp16 recompute of out rows 0..127: little attention averaging
            there, so fp8 value-chain noise is too big for those rows."""
            if 0 not in wo_stage:
                wo_stage[0] = (
                    [stpool.tile([128, 2 * 512], FP16, name=f"pst0_{q}",
                                 tag=f"pst{q}") for q in range(4)],
                    drpool.tile([D_MODEL, 512], FP16, name="prs0",
                                tag="prs0"))
            pstage_q, prs = wo_stage[0]
            pv = ps_m.tile([128, 512], F32, name="pv16", tag="m")
            for dj in range(NT_D):
                nc.tensor.matmul(
                    pv[:, 0:CSL], xT16v[:, dj, :], wv16v[:, dj, :],
                    start=(dj == 0), stop=(dj == NT_D - 1),
                    skip_group_check=True)
            vt16 = vpool.tile([128, HPC * 65], FP16, name="vt16", tag="vt16")
            v16v = vt16.rearrange("p (h e) -> p h e", e=65)
            nc.vector.memset(v16v[:, :, 64:65], 1.0)
            nc.vector.tensor_add(
                v16v[:, :, 0:64],
                pv[:, 0:CSL].rearrange("p (h e) -> p h e", e=64),
                bv16.rearrange("p (h e) -> p h e", e=64))
            pa16 = ps_att.tile([65, 512], F32, name="pa16", tag="att0")
            for ci in range(2):
                ps16 = ps_pair.tile([128, 1024], F32, name="ps16", tag="sp")
                psv16 = ps16.rearrange("p (h c) -> p h c", h=2)
                for hh in range(2):
                    p0 = 64 * hh
                    nc.tensor.matmul(
                        ps16[:, 512 * hh:512 * hh + 128],
                        kt[ci][p0:p0 + 64, 0:128], qt[ci][p0:p0 + 64, 0:128],
                        start=True, stop=False)
                nc.tensor.matmul(
                    psv16[:, :, 0:128],
                    id16[:], negm16[:],
                    start=False, stop=True, skip_group_check=True)
                et16 = misc_c.tile([128, 256], FP16, name="et16", tag="et16")
                nc.scalar.activation(
                    et16.rearrange("p (h c) -> p h c", c=128)[:, :, :],
                    psv16[:, :, 0:128], AF.Exp)
                for hh in range(2):
                    hl = 2 * ci + hh
                    nc.tensor.matmul(
                        pa16[:, 128 * hl:128 * (hl + 1)],
                        vt16[:, 65 * hl:65 * hl + 65],
                        et16[:, 128 * hh:128 * (hh + 1)],
                        start=(hl == 0), stop=(hl == 3),
                        skip_group_check=True)
            rt16 = misc_c.tile([1, 512], FP16, name="rt16", tag="rt16")
            with nc.allow_low_precision(reason="softmax 1/denom in fp16"):
                nc.vector.reciprocal(rt16[:], pa16[64:65, 0:512])
            pb16 = ps_m.tile([128, 512], F32, name="pb16", tag="m")
            nc.tensor.matmul(pb16[0:64, :], ones16[0:1, 0:64], rt16[:],
                             start=True, stop=True)
            bc16 = misc_c.tile([128, 512], FP16, name="bc16", tag="bc16")
            nc.vector.tensor_scalar_add(bc16[0:64, :], pb16[0:64, :], 0.0)
            h16 = htpool.tile([128, 2 * 128], FP16, name="h16", tag="h16")
            h16v = h16.rearrange("p (s c) -> p s c", c=128)
            for ci in range(2):
                for hh in range(2):
                    hl = 2 * ci + hh
                    nc.vector.tensor_mul(
                        h16v[64 * hh:64 * (hh + 1), ci, :],
                        pa16[0:64, 128 * hl:128 * (hl + 1)],
                        bc16[0:64, 128 * hl:128 * (hl + 1)])
            for t in range(NT_D):
                po = ps_m.tile([128, 512], F32, name="po16", tag="m")
                for s in range(2):
                    nc.tensor.matmul(
                        po[:, 0:128],
                        wo16v[:, s, 128 * t:128 * (t + 1)],
                        h16v[:, s, :],
                        start=(s == 0), stop=(s == 1), skip_group_check=True)
                nc.vector.tensor_scalar(
                    pstage_q[t // 2].rearrange(
                        "p (t c) -> p t c", c=512)[:, t % 2, 0:128],
                    po[:, 0:128],
                    1.0, boT4[:, t:t + 1], op0=MUL, op1=ADD)

        # ---- pipeline over q-chunks ----
        # chunk-0 q/k run dj-major so PE consumes x slices as they arrive
        qk0 = [(wqv, 0, qt, 0), (wkv, 2, kt, 0),
               (wqv, 0, qt, 1), (wkv, 2, kt, 1)]
        pps0 = []
        for i in range(2):
            t = ps_pair.tile([128, 1024], F32, name=f"pp0_{i}", tag="sp")
            pps0 += [t[:, 0:512], t[:, 512:1024]]
        # ci0's q/k eagerly (attention ci0 starts as soon as they land);
        # everything else chunk-0 interleaves into attention(0) as fillers
        for dj in range(0, NT_D, 2):
            for u in (0, 1):
                wv_, bcol, dst, ci = qk0[u]
                nc.tensor.matmul(
                    pps0[u],
                    wv_[:, dj:dj + 2, 128 * ci:128 * (ci + 1)],
                    xTv[:, dj:dj + 2, 0:512],
                    start=(dj == 0), stop=(dj == 6),
                    perf_mode=DR, skip_group_check=True)
        for u in (0, 1):
            wv_, bcol, dst, ci = qk0[u]
            nc.vector.tensor_scalar(
                dst[ci][:, 0:512], pps0[u],
                1.0 / WS, bqk[:, bcol + ci:bcol + ci + 1],
                op0=MUL, op1=ADD)
        for sl in (2, 3):
            st = {}
            for half in range(2):
                pending.append(
                    lambda l=sl, h=half, t=st: emit_v_half(0, l, h, t))
        for u in (2, 3):
            wv_, bcol, dst, ci = qk0[u]
            st = {}

            def qk0_half(h, u=u, st=st):
                wv_, bcol, dst, ci = qk0[u]
                if h == 0:
                    st["pp"] = pps0[u]
                for dj in ((0, 2) if h == 0 else (4, 6)):
                    nc.tensor.matmul(
                        pps0[u],
                        wv_[:, dj:dj + 2, 128 * ci:128 * (ci + 1)],
                        xTv[:, dj:dj + 2, 0:512],
                        start=(dj == 0), stop=(dj == 6),
                        perf_mode=DR, skip_group_check=True)
                if h == 1:
                    nc.vector.tensor_scalar(
                        dst[ci][:, 0:512], pps0[u],
                        1.0 / WS, bqk[:, bcol + ci:bcol + ci + 1],
                        op0=MUL, op1=ADD)
            for half in range(2):
                pending.append(lambda h=half, f=qk0_half: f(h))
        for sl in (0, 1):
            st = {}
            for half in range(2):
                pending.append(
                    lambda l=sl, h=half, t=st: emit_v_half(0, l, h, t))
        for sj in range(NQ):
            if sj > 0 and sj - 1 not in wo_done:
                for u in range(NT_D):
                    pending_wo.append(lambda s=sj - 1, t=u: emit_wo_t(s, t))
            if sj + 1 < NQ:
                nsj = sj + 1
                for ci in range(2):
                    for wv_, bcol, dst in [(wqv, 0, qt), (wkv, 2, kt)]:
                        st = {}
                        for half in range(2):
                            pending.append(
                                lambda c=ci, s=nsj, w=wv_, b=bcol, d=dst,
                                h=half, t=st: emit_qk_half(s, c, w, b, d, h, t))
                for sl in range(4):
                    st = {}
                    for half in range(2):
                        pending.append(
                            lambda s=nsj, l=sl, h=half, t=st:
                            emit_v_half(s, l, h, t))
            emit_attention(sj)
            pop_pending(len(pending))
            if sj == 0:
                flush_norm()
                emit_side()
            if sj == NQ - 2:
                # inline the chunk-2 norm+wo so RS2 starts early enough that
                # the collective chain is off the critical path
                flush_norm()
                for t in range(NT_D):
                    emit_wo_t(sj, t)
                wo_done.add(sj)
        flush_norm()
        for t in range(NT_D):
            emit_wo_t(NQ - 1, t)
        for f in out_dmas:
            f()


def _consts():
    kk = np.arange(128)[:, None]
    qq = np.arange(128)[None, :]
    negm16 = np.where(kk <= qq, 0.0, -60000.0).astype(np.float16)
    negm16 = np.concatenate([negm16, negm16], axis=1)
    id16 = np.eye(128, dtype=np.float16)
    return negm16, id16


F8NP = ml_dtypes.float8_e4m3


def _to8(a):
    return np.clip(a, -224.0, 224.0).astype(F8NP)


def _pack_w(w):
    """[1024, C] -> [128, 8*C] with d-tile t at column block t."""
    c = w.shape[1]
    return np.ascontiguousarray(
        w.reshape(NT_D, 128, c).transpose(1, 0, 2).reshape(128, NT_D * c))


def kernel(x, Wq, bq, Wk, bk, Wv, bv, Wo, bo):
    x = np.asarray(x, dtype=np.float32)
    Wq = np.asarray(Wq, dtype=np.float32)
    bq = np.asarray(bq, dtype=np.float32)
    Wk = np.asarray(Wk, dtype=np.float32)
    bk = np.asarray(bk, dtype=np.float32)
    Wv = np.asarray(Wv, dtype=np.float32)
    bv = np.asarray(bv, dtype=np.float32)
    Wo = np.asarray(Wo, dtype=np.float32)
    bo = np.asarray(bo, dtype=np.float32)

    if "nc" not in _cache:
        _cache["nc"] = _build()
    nc = _cache["nc"]

    negm16, id16 = _consts()
    scale = 1.0 / np.sqrt(np.float32(D_HEAD))
    in_maps = []
    for core in range(N_CORES):
        b, g = divmod(core, HPC)
        csl = slice(CSL * g, CSL * (g + 1))
        wo_loc = Wo[csl, :]  # [256, 1024]
        wo_pack = np.ascontiguousarray(
            wo_loc.reshape(2, 128, D_MODEL).transpose(1, 0, 2).reshape(
                128, 2 * D_MODEL))
        bqk = np.stack([
            bq[csl][0:128] * scale, bq[csl][128:256] * scale,
            bk[csl][0:128], bk[csl][128:256],
        ], axis=1).astype(np.float32)
        wqkv = np.concatenate([
            _pack_w(Wq[:, csl] * (scale * WS)),
            _pack_w(Wk[:, csl] * WS),
            _pack_w(Wv[:, csl] * WS),
            wo_pack * WS,
        ], axis=1)
        cst32 = np.concatenate([
            bqk,
            np.broadcast_to(WS * bv[None, csl], (128, CSL)),
            (bo / HPC).reshape(NT_D, 128).T,
            np.broadcast_to(bv[None, csl], (128, CSL)),
        ], axis=1).astype(np.float32)
        borow = np.zeros((128, 1024), dtype=np.float32)
        borow[0, :] = (WS * WS / HPC) * bo
        cst16 = np.concatenate([
            negm16.astype(np.float32), id16.astype(np.float32),
            _pack_w(Wv[:, csl]),
            wo_pack,
            _pack_w(np.ascontiguousarray(x[b].T[:, 0:128])),
            borow,
        ], axis=1).astype(np.float16)
        in_maps.append({
            "xt": _to8(np.ascontiguousarray(x[b].T)),
            "wqkv": _to8(wqkv),
            "cst32": np.ascontiguousarray(cst32),
            "cst16": np.ascontiguousarray(cst16),
        })

    # the axon terminal occasionally reports a transient
    # NRT_EXEC_UNIT_UNRECOVERABLE; retries with backoff recover it
    import time as _time
    for attempt in range(3):
        try:
            res = bass_utils.run_bass_kernel_spmd(
                nc, in_maps, core_ids=list(range(N_CORES)))
            break
        except Exception:
            if attempt == 2:
                raise
            _time.sleep(5.0 * (attempt + 1))

    full = np.empty((BATCH, SEQ, D_MODEL), dtype=np.float32)
    for core in range(N_CORES):
        b, g = divmod(core, HPC)
        o = np.asarray(res.results[core]["out"]).astype(np.float32)
        for sj in range(NQ):
            full[b, 512 * sj:512 * (sj + 1), 256 * g:256 * (g + 1)] = \
                o[256 * sj:256 * (sj + 1), :].T
    return full


# revision 40
# speedup vs baseline: 1.2197x; 1.0128x over previous
import sys

sys.path.insert(0, '/opt/trn_rl_repo')

import ml_dtypes
import numpy as np
import concourse.bass as bass
import concourse.mybir as mybir
import concourse.tile as tile
from concourse import bacc, bass_utils

F32 = mybir.dt.float32
BF16 = mybir.dt.bfloat16
FP16 = mybir.dt.float16
FP8 = mybir.dt.float8e4
AF = mybir.ActivationFunctionType
DR = mybir.MatmulPerfMode.DoubleRow
MUL = mybir.AluOpType.mult
ADD = mybir.AluOpType.add

D_MODEL = 1024
N_HEADS = 16
D_HEAD = 64
SEQ = 2048
BATCH = 2
N_CORES = 8
HPC = 4             # heads per core
CSL = HPC * D_HEAD  # 256: qkv feature slice per core
NT_D = D_MODEL // 128  # 8
NQ = SEQ // 512     # 4 q-chunks
GROUPS = [[0, 1, 2, 3], [4, 5, 6, 7]]
WS = 32.0           # fp8 weight pre-scale (host side)

_cache = {}


def _build():
    nc = bacc.Bacc("TRN2", target_bir_lowering=False, debug=False,
                   num_devices=N_CORES)
    xt_in = nc.dram_tensor("xt", [D_MODEL, SEQ], FP8, kind="ExternalInput").ap()
    w_in = nc.dram_tensor("wqkv", [128, 4 * NT_D * CSL], FP8,
                          kind="ExternalInput").ap()
    c32_in = nc.dram_tensor("cst32", [128, 4 + 2 * CSL + NT_D], F32,
                            kind="ExternalInput").ap()
    c16_in = nc.dram_tensor("cst16", [128, 384 + 2 * NT_D * CSL + 2048], FP16,
                            kind="ExternalInput").ap()
    out = nc.dram_tensor("out", [NQ * 256, 512], FP16,
                         kind="ExternalOutput").ap()

    with tile.TileContext(nc) as tc:
        _body(nc, tc, xt_in, w_in, c32_in, c16_in, out)
    nc.compile()
    return nc


def _body(nc, tc, xt_in, w_in, c32_in, c16_in, out):
    from contextlib import ExitStack
    ctx = ExitStack()
    with ctx:
        const = ctx.enter_context(tc.tile_pool(name="const", bufs=1))
        wpool = ctx.enter_context(tc.tile_pool(name="wpool", bufs=1))
        xtpool = ctx.enter_context(tc.tile_pool(name="xtpool", bufs=1))
        qkpool = ctx.enter_context(tc.tile_pool(name="qkpool", bufs=1))
        vpool = ctx.enter_context(tc.tile_pool(name="vpool", bufs=1))
        htpool = ctx.enter_context(tc.tile_pool(name="htpool", bufs=1))
        exp_pool = ctx.enter_context(tc.tile_pool(name="exp_pool", bufs=6))
        misc_c = ctx.enter_context(tc.tile_pool(name="misc_c", bufs=2))
        stpool = ctx.enter_context(tc.tile_pool(name="stpool", bufs=2))
        drpool = ctx.enter_context(tc.tile_pool(name="drpool", bufs=1, space="DRAM"))
        ps_pair = ctx.enter_context(tc.tile_pool(name="ps_pair", bufs=2, space="PSUM"))
        ps_att = ctx.enter_context(tc.tile_pool(name="ps_att", bufs=1, space="PSUM"))
        ps_m = ctx.enter_context(tc.tile_pool(name="ps_m", bufs=2, space="PSUM"))

        # ---- startup: few fat DMAs (HWDGE desc-gen is ~630ns each) ----
        xT = xtpool.tile([128, NT_D * SEQ], FP8, name="xT", tag="xT")
        xTv = xT.rearrange("p (d s) -> p d s", s=SEQ)
        xdr = xt_in.rearrange("(d p) s -> p d s", p=128)
        nc.sync.dma_start(xTv[:, 0:4, 0:512], xdr[:, 0:4, 0:512])
        wt = wpool.tile([128, 4 * NT_D * CSL], FP8, name="w_all")
        nc.scalar.dma_start(wt[:, 0:NT_D * CSL], w_in[:, 0:NT_D * CSL])
        nc.sync.dma_start(xTv[:, 4:8, 0:512], xdr[:, 4:8, 0:512])
        nc.scalar.dma_start(wt[:, NT_D * CSL:2 * NT_D * CSL],
                            w_in[:, NT_D * CSL:2 * NT_D * CSL])
        c16 = const.tile([128, 384 + 2 * NT_D * CSL + 2048], FP16)
        nc.scalar.dma_start(c16[:, 0:384], c16_in[:, 0:384])
        c32 = const.tile([128, 4 + 2 * CSL + NT_D], F32)
        nc.scalar.dma_start(c32[:], c32_in[:])
        nc.scalar.dma_start(wt[:, 2 * NT_D * CSL:], w_in[:, 2 * NT_D * CSL:])
        nc.sync.dma_start(xTv[:, :, 512:SEQ], xdr[:, :, 512:SEQ])
        nc.scalar.dma_start(c16[:, 384:], c16_in[:, 384:])
        wq_t = wt[:, 0:NT_D * CSL]
        wk_t = wt[:, NT_D * CSL:2 * NT_D * CSL]
        wv_t = wt[:, 2 * NT_D * CSL:3 * NT_D * CSL]
        wo_t = wt[:, 3 * NT_D * CSL:4 * NT_D * CSL]
        bqk = c32[:, 0:4]
        bv32 = c32[:, 4:4 + CSL]
        boT4 = c32[:, 4 + CSL:4 + CSL + NT_D]
        negm16 = c16[:, 0:256]
        id16 = c16[:, 256:384]
        wv16 = c16[:, 384:384 + NT_D * CSL]
        wv16v = wv16.rearrange("p (d m) -> p d m", m=CSL)
        wo16v = c16[:, 384 + NT_D * CSL:384 + 2 * NT_D * CSL].rearrange(
            "p (s c) -> p s c", c=D_MODEL)
        xT16v = c16[:, 384 + 2 * NT_D * CSL:].rearrange(
            "p (d s) -> p d s", s=128)
        bv16 = c32[:, 4 + CSL + NT_D:4 + 2 * CSL + NT_D]
        ones16 = const.tile([1, 128], FP16)
        nc.vector.memset(ones16[:], 1.0)
        actwarm = const.tile([1, 4], FP16)
        nc.vector.memset(actwarm[:], 0.0)
        nc.scalar.activation(actwarm[:], actwarm[:], AF.Exp)
        onesw = const.tile([1, 512], FP16)
        nc.vector.memset(onesw[:], 1.0)
        bo16 = c16[0:1, 384 + 2 * NT_D * CSL + 1024:]

        # ---- persistent activations ----
        qt, kt = [], []
        for ci in range(2):
            qt.append(qkpool.tile([128, SEQ], FP16, name=f"qt{ci}", tag=f"qt{ci}"))
            kt.append(qkpool.tile([128, SEQ], FP16, name=f"kt{ci}", tag=f"kt{ci}"))
        hT = htpool.tile([128, 2 * SEQ], FP8, name="hT", tag="hT")
        hTv = hT.rearrange("p (s c) -> p s c", c=SEQ)
        vtp = [vpool.tile([128, 2 * HPC * 68], FP8, name=f"vt{j}", tag=f"vt{j}")
               for j in range(NT_D)]
        wav = wt.rearrange("p (w d m) -> p w d m", w=4, m=CSL)
        wqv = wav[:, 0]
        wkv = wav[:, 1]
        wvv = wav[:, 2]
        wov = wt.rearrange("p (w s c) -> p w s c", w=4, c=D_MODEL)[:, 3]

        def emit_qk_half(sj, ci, wv_, bcol, dst, half, state):
            """Half a q/k projection slice: 2 DR matmuls (+ bias on 2nd)."""
            if half == 0:
                state["pp"] = ps_m.tile([128, 512], F32, name="pp", tag="m")
            pp = state["pp"]
            for dj in (4 * half, 4 * half + 2):
                nc.tensor.matmul(
                    pp[:],
                    wv_[:, dj:dj + 2, 128 * ci:128 * (ci + 1)],
                    xTv[:, dj:dj + 2, 512 * sj:512 * (sj + 1)],
                    start=(dj == 0), stop=(dj == 6),
                    perf_mode=DR, skip_group_check=True)
            if half == 1:
                nc.vector.tensor_scalar(
                    dst[ci][:, 512 * sj:512 * (sj + 1)], pp[:],
                    1.0 / WS, bqk[:, bcol + ci:bcol + ci + 1],
                    op0=MUL, op1=ADD)

        def emit_qk(sj, ci, wv_, bcol, dst):
            state = {}
            emit_qk_half(sj, ci, wv_, bcol, dst, 0, state)
            emit_qk_half(sj, ci, wv_, bcol, dst, 1, state)

        def emit_v_half(sj, sl, half, state):
            """Half a v s-tile: 2 DR matmuls (+ ones memset and add on 2nd)."""
            si = 4 * sj + sl
            if half == 0:
                state["pv"] = ps_m.tile([128, 512], F32, name="pv", tag="m")
            pv = state["pv"]
            for dj in (4 * half, 4 * half + 2):
                nc.tensor.matmul(
                    pv[:, 0:CSL],
                    xTv[:, dj:dj + 2, 128 * si:128 * (si + 1)],
                    wvv[:, dj:dj + 2, :],
                    start=(dj == 0), stop=(dj == 6),
                    perf_mode=DR, skip_group_check=True)
            if half == 1:
                j, slot = divmod(si, 2)
                vv = vtp[j].rearrange("p (s h e) -> p s h e", s=2, e=68)
                nc.vector.memset(vv[:, slot, :, 64:65], 1.0)
                nc.vector.tensor_add(
                    vv[:, slot, :, 0:64],
                    pv[:, 0:CSL].rearrange("p (h e) -> p h e", e=64),
                    bv32.rearrange("p (h e) -> p h e", e=64))

        def emit_v(sj, sl):
            state = {}
            emit_v_half(sj, sl, 0, state)
            emit_v_half(sj, sl, 1, state)

        pending = []
        pending_wo = []

        def pop_pending(n):
            for _ in range(min(n, len(pending))):
                pending.pop(0)()

        def pop_wo(n=100):
            for _ in range(min(n, len(pending_wo))):
                pending_wo.pop(0)()

        deferred = [None]

        def flush_norm():
            if deferred[0] is not None:
                f = deferred[0]
                deferred[0] = None
                f()

        def emit_norm(sj, ci, pa):
            # softmax denominators -> reciprocal -> broadcast -> scale
            rt = misc_c.tile([1, 1024], FP16, name="rt", tag="rt")
            with nc.allow_low_precision(reason="softmax 1/denom in fp16"):
                nc.vector.reciprocal(rt[:, 0:512], pa[0][64:65, :])
                nc.vector.reciprocal(rt[:, 512:1024], pa[1][64:65, :])
            pb = ps_m.tile([128, 512], F32, name="pb", tag="m")
            nc.tensor.matmul(pb[0:64, :], ones16[0:1, 0:64], rt[:, 0:512],
                             start=True, stop=True)
            nc.tensor.matmul(pb[64:128, :], ones16[0:1, 0:64], rt[:, 512:1024],
                             start=True, stop=True, tile_position=(0, 64))
            bc = misc_c.tile([128, 512], FP16, name="bc", tag="bc")
            nc.vector.tensor_scalar_add(bc[:], pb[:], 0.0)
            pop_pending(2)  # cover the bc wait with filler matmuls
            for hh in range(2):
                nc.vector.tensor_mul(
                    hTv[64 * hh:64 * (hh + 1), ci, 512 * sj:512 * (sj + 1)],
                    pa[hh][0:64, :], bc[64 * hh:64 * (hh + 1), :])

        def emit_attention(sj):
            npairs = 2 * sj + 2
            # diag pairs first (fast psum refill), then off-diag ascending
            order = [2 * sj + 1, 2 * sj] + list(range(2 * sj))
            for ci in range(2):
                pa = [ps_att.tile([65, 512], F32, name=f"pa{hh}", tag=f"att{hh}")
                      for hh in range(2)]

                def emit_av(j, etv, pos):
                    """attn@V for pair j (software-pipelined one pair behind)."""
                    is_diag_hi = (j == 2 * sj + 1)
                    is_diag_lo = (j == 2 * sj)
                    last = (pos == npairs - 1)
                    for hh in range(2):
                        h_local = 2 * ci + hh
                        vv = vtp[j].rearrange(
                            "p (s h e) -> p s h e", s=2, e=68)[:, :, h_local, 0:65]
                        if is_diag_hi:
                            # tiles r2 (c0=256), r3 (c0=384)
                            nc.tensor.matmul(
                                pa[hh][:, 384:512], vv, etv[:, hh, :, 384:512],
                                start=(pos == 0), stop=False,
                                perf_mode=DR, skip_group_check=True)
                            nc.tensor.matmul(
                                pa[hh][:, 256:384], vv[:, 0, :],
                                etv[:, hh, 0, 256:384],
                                start=False, stop=False, skip_group_check=True)
                        elif is_diag_lo:
                            # tiles r0 (c0=0), r1 (c0=128)
                            nc.tensor.matmul(
                                pa[hh][:, 128:512], vv, etv[:, hh, :, 128:512],
                                start=False, stop=False,
                                perf_mode=DR, skip_group_check=True)
                            nc.tensor.matmul(
                                pa[hh][:, 0:128], vv[:, 0, :],
                                etv[:, hh, 0, 0:128],
                                start=False, stop=last, skip_group_check=True)
                        else:
                            nc.tensor.matmul(
                                pa[hh][:, 0:512], vv, etv[:, hh, :, 0:512],
                                start=False, stop=last,
                                perf_mode=DR, skip_group_check=True)

                prev = None
                prev2 = None
                prev3 = None
                for pos, j in enumerate(order):
                    et = exp_pool.tile([128, 2048], FP8, name="et", tag="et")
                    etv = et.rearrange("p (h s c) -> p h s c", h=2, c=512)
                    for slot in range(2):
                        ki = 2 * j + slot
                        r = ki - 4 * sj
                        c0 = 0 if r < 0 else 128 * r
                        ps = ps_pair.tile([128, 1024], F32, name="ps", tag="sp")
                        psv = ps.rearrange("p (h c) -> p h c", h=2)
                        for hh in range(2):
                            p0 = 64 * hh
                            nc.tensor.matmul(
                                ps[:, 512 * hh + c0:512 * (hh + 1)],
                                kt[ci][p0:p0 + 64, 128 * ki:128 * (ki + 1)],
                                qt[ci][p0:p0 + 64, 512 * sj + c0:512 * (sj + 1)],
                                start=True, stop=(r < 0))
                        if r >= 0:
                            # causal mask via PE, both heads
                            nc.tensor.matmul(
                                psv[:, :, c0:c0 + 128],
                                id16[:], negm16[:],
                                start=False, stop=True, skip_group_check=True)
                        nc.scalar.activation(
                            etv[:, :, slot, c0:512],
                            psv[:, :, c0:512],
                            AF.Exp)
                    if pos == 0:
                        flush_norm()
                        pop_wo(2)
                        if sj == 0 and ci == 0:
                            pop_pending(4)
                    if prev3 is not None:
                        emit_av(*prev3)
                        pop_wo(2)
                        pop_pending(1)
                    prev3 = prev2
                    prev2 = prev
                    prev = (j, etv, pos)
                if sj == 0 and ci == 0:
                    pop_pending(8)
                for pv in (prev3, prev2, prev):
                    if pv is not None:
                        emit_av(*pv)
                deferred[0] = lambda s=sj, c=ci, p=pa: emit_norm(s, c, p)
                pop_pending(1)

        wo_stage = {}
        wo_done = set()
        out_dmas = []

        def emit_wo_t(sj, t):
            """One transposed out block [128 dout, 512 seq] = Wo_t.T @ hT."""
            if sj not in wo_stage:
                wo_stage[sj] = (
                    [stpool.tile([128, 2 * 512], FP16, name=f"pst{sj}_{q}",
                                 tag=f"pst{q}") for q in range(4)],
                    drpool.tile([D_MODEL, 512], FP16, name=f"prs{sj}",
                                tag=f"prs{sj}"))
            pstage, prs = wo_stage[sj]
            pstage = pstage[t // 2]
            po = ps_m.tile([128, 512], F32, name="po", tag="m")
            tail = (sj == NQ - 1)
            nc.tensor.matmul(
                po[:],
                wov[:, :, 128 * t:128 * (t + 1)],
                hTv[:, :, 512 * sj:512 * (sj + 1)],
                start=True, stop=not tail, perf_mode=DR, skip_group_check=True)
            c0w = 128 if sj == 0 else 0
            dst = pstage.rearrange("p (t c) -> p t c", c=512)[:, t % 2, c0w:512]
            if tail:
                # bias via PE; staging split ACT/DVE (ACT idles at the tail)
                nc.tensor.matmul(
                    po[:], bo16[0:1, 128 * t:128 * (t + 1)], onesw[0:1, :],
                    start=False, stop=True, skip_group_check=True)
                if t % 2 == 0:
                    nc.scalar.mul(dst, po[:, c0w:512], 1.0 / (WS * WS))
                else:
                    nc.vector.tensor_scalar(
                        dst, po[:, c0w:512], 1.0 / (WS * WS), None, op0=MUL)
            else:
                nc.vector.tensor_scalar(
                    dst, po[:, c0w:512],
                    1.0 / (WS * WS), boT4[:, t:t + 1], op0=MUL, op1=ADD)
            if t % 2 == 1:
                eng = nc.scalar if sj == NQ - 1 else nc.sync
                eng.dma_start(
                    prs.rearrange("(t p) c -> p t c", p=128)[:, t - 1:t + 1, :],
                    pstage.rearrange("p (t c) -> p t c", c=512)[:, :, :])
            if t == NT_D - 1:
                pro = drpool.tile([256, 512], FP16, name=f"pro{sj}",
                                  tag=f"pro{sj}")
                nc.gpsimd.collective_compute(
                    "ReduceScatter", mybir.AluOpType.add,
                    replica_groups=GROUPS, ins=[prs[:]], outs=[pro[:]])
                out_dmas.append(
                    lambda s=sj, p=pro: nc.sync.dma_start(
                        out[256 * s:256 * (s + 1), :], p[:]))

        def emit_side():
            """fp16 recompute of out rows 0..127: little attention averaging
            there, so fp8 value-chain noise is too big for those rows."""
            if 0 not in wo_stage:
                wo_stage[0] = (
                    [stpool.tile([128, 2 * 512], FP16, name=f"pst0_{q}",
                                 tag=f"pst{q}") for q in range(4)],
                    drpool.tile([D_MODEL, 512], FP16, name="prs0",
                                tag="prs0"))
            pstage_q, prs = wo_stage[0]
            pv = ps_m.tile([128, 512], F32, name="pv16", tag="m")
            for dj in range(NT_D):
                nc.tensor.matmul(
                    pv[:, 0:CSL], xT16v[:, dj, :], wv16v[:, dj, :],
                    start=(dj == 0), stop=(dj == NT_D - 1),
                    skip_group_check=True)
            vt16 = vpool.tile([128, HPC * 65], FP16, name="vt16", tag="vt16")
            v16v = vt16.rearrange("p (h e) -> p h e", e=65)
            nc.vector.memset(v16v[:, :, 64:65], 1.0)
            nc.vector.tensor_add(
                v16v[:, :, 0:64],
                pv[:, 0:CSL].rearrange("p (h e) -> p h e", e=64),
                bv16.rearrange("p (h e) -> p h e", e=64))
            pa16 = ps_att.tile([65, 512], F32, name="pa16", tag="att0")
            for ci in range(2):
                ps16 = ps_pair.tile([128, 1024], F32, name="ps16", tag="sp")
                psv16 = ps16.rearrange("p (h c) -> p h c", h=2)
                for hh in range(2):
                    p0 = 64 * hh
                    nc.tensor.matmul(
                        ps16[:, 512 * hh:512 * hh + 128],
                        kt[ci][p0:p0 + 64, 0:128], qt[ci][p0:p0 + 64, 0:128],
                        start=True, stop=False)
                nc.tensor.matmul(
                    psv16[:, :, 0:128],
                    id16[:], negm16[:],
                    start=False, stop=True, skip_group_check=True)
                et16 = misc_c.tile([128, 256], FP16, name="et16", tag="et16")
                nc.scalar.activation(
                    et16.rearrange("p (h c) -> p h c", c=128)[:, :, :],
                    psv16[:, :, 0:128], AF.Exp)
                for hh in range(2):
                    hl = 2 * ci + hh
                    nc.tensor.matmul(
                        pa16[:, 128 * hl:128 * (hl + 1)],
                        vt16[:, 65 * hl:65 * hl + 65],
                        et16[:, 128 * hh:128 * (hh + 1)],
                        start=(hl == 0), stop=(hl == 3),
                        skip_group_check=True)
            rt16 = misc_c.tile([1, 512], FP16, name="rt16", tag="rt16")
            with nc.allow_low_precision(reason="softmax 1/denom in fp16"):
                nc.vector.reciprocal(rt16[:], pa16[64:65, 0:512])
            pb16 = ps_m.tile([128, 512], F32, name="pb16", tag="m")
            nc.tensor.matmul(pb16[0:64, :], ones16[0:1, 0:64], rt16[:],
                             start=True, stop=True)
            bc16 = misc_c.tile([128, 512], FP16, name="bc16", tag="bc16")
            nc.vector.tensor_scalar_add(bc16[0:64, :], pb16[0:64, :], 0.0)
            h16 = htpool.tile([128, 2 * 128], FP16, name="h16", tag="h16")
            h16v = h16.rearrange("p (s c) -> p s c", c=128)
            for ci in range(2):
                for hh in range(2):
                    hl = 2 * ci + hh
                    nc.vector.tensor_mul(
                        h16v[64 * hh:64 * (hh + 1), ci, :],
                        pa16[0:64, 128 * hl:128 * (hl + 1)],
                        bc16[0:64, 128 * hl:128 * (hl + 1)])
            for t in range(NT_D):
                po = ps_m.tile([128, 512], F32, name="po16", tag="m")
                for s in range(2):
                    nc.tensor.matmul(
                        po[:, 0:128],
                        wo16v[:, s, 128 * t:128 * (t + 1)],
                        h16v[:, s, :],
                        start=(s == 0), stop=(s == 1), skip_group_check=True)
                nc.vector.tensor_scalar(
                    pstage_q[t // 2].rearrange(
                        "p (t c) -> p t c", c=512)[:, t % 2, 0:128],
                    po[:, 0:128],
                    1.0, boT4[:, t:t + 1], op0=MUL, op1=ADD)

        # ---- pipeline over q-chunks ----
        # chunk-0 q/k run dj-major so PE consumes x slices as they arrive
        qk0 = [(wqv, 0, qt, 0), (wkv, 2, kt, 0),
               (wqv, 0, qt, 1), (wkv, 2, kt, 1)]
        pps0 = []
        for i in range(2):
            t = ps_pair.tile([128, 1024], F32, name=f"pp0_{i}", tag="sp")
            pps0 += [t[:, 0:512], t[:, 512:1024]]
        # ci0's q/k eagerly (attention ci0 starts as soon as they land);
        # everything else chunk-0 interleaves into attention(0) as fillers
        for dj in range(0, NT_D, 2):
            for u in (0, 1):
                wv_, bcol, dst, ci = qk0[u]
                nc.tensor.matmul(
                    pps0[u],
                    wv_[:, dj:dj + 2, 128 * ci:128 * (ci + 1)],
                    xTv[:, dj:dj + 2, 0:512],
                    start=(dj == 0), stop=(dj == 6),
                    perf_mode=DR, skip_group_check=True)
        for u in (0, 1):
            wv_, bcol, dst, ci = qk0[u]
            nc.vector.tensor_scalar(
                dst[ci][:, 0:512], pps0[u],
                1.0 / WS, bqk[:, bcol + ci:bcol + ci + 1],
                op0=MUL, op1=ADD)
        for sl in (2, 3):
            st = {}
            for half in range(2):
                pending.append(
                    lambda l=sl, h=half, t=st: emit_v_half(0, l, h, t))
        for u in (2, 3):
            wv_, bcol, dst, ci = qk0[u]
            st = {}

            def qk0_half(h, u=u, st=st):
                wv_, bcol, dst, ci = qk0[u]
                if h == 0:
                    st["pp"] = pps0[u]
                for dj in ((0, 2) if h == 0 else (4, 6)):
                    nc.tensor.matmul(
                        pps0[u],
                        wv_[:, dj:dj + 2, 128 * ci:128 * (ci + 1)],
                        xTv[:, dj:dj + 2, 0:512],
                        start=(dj == 0), stop=(dj == 6),
                        perf_mode=DR, skip_group_check=True)
                if h == 1:
                    nc.vector.tensor_scalar(
                        dst[ci][:, 0:512], pps0[u],
                        1.0 / WS, bqk[:, bcol + ci:bcol + ci + 1],
                        op0=MUL, op1=ADD)
            for half in range(2):
                pending.append(lambda h=half, f=qk0_half: f(h))
        for sl in (0, 1):
            st = {}
            for half in range(2):
                pending.append(
                    lambda l=sl, h=half, t=st: emit_v_half(0, l, h, t))
        for sj in range(NQ):
            if sj > 0 and sj - 1 not in wo_done:
                for u in range(NT_D):
                    pending_wo.append(lambda s=sj - 1, t=u: emit_wo_t(s, t))
            if sj + 1 < NQ:
                nsj = sj + 1
                for ci in range(2):
                    for wv_, bcol, dst in [(wqv, 0, qt), (wkv, 2, kt)]:
                        st = {}
                        for half in range(2):
                            pending.append(
                                lambda c=ci, s=nsj, w=wv_, b=bcol, d=dst,
                                h=half, t=st: emit_qk_half(s, c, w, b, d, h, t))
                for sl in range(4):
                    st = {}
                    for half in range(2):
                        pending.append(
                            lambda s=nsj, l=sl, h=half, t=st:
                            emit_v_half(s, l, h, t))
            emit_attention(sj)
            pop_pending(len(pending))
            if sj == 0:
                flush_norm()
                emit_side()
            if sj == NQ - 2:
                # inline the chunk-2 norm+wo so RS2 starts early enough that
                # the collective chain is off the critical path
                flush_norm()
                for t in range(NT_D):
                    emit_wo_t(sj, t)
                wo_done.add(sj)
        flush_norm()
        for t in range(NT_D):
            emit_wo_t(NQ - 1, t)
        for f in out_dmas:
            f()


def _consts():
    kk = np.arange(128)[:, None]
    qq = np.arange(128)[None, :]
    negm16 = np.where(kk <= qq, 0.0, -60000.0).astype(np.float16)
    negm16 = np.concatenate([negm16, negm16], axis=1)
    id16 = np.eye(128, dtype=np.float16)
    return negm16, id16


F8NP = ml_dtypes.float8_e4m3


def _to8(a):
    return np.clip(a, -224.0, 224.0).astype(F8NP)


def _pack_w(w):
    """[1024, C] -> [128, 8*C] with d-tile t at column block t."""
    c = w.shape[1]
    return np.ascontiguousarray(
        w.reshape(NT_D, 128, c).transpose(1, 0, 2).reshape(128, NT_D * c))


def kernel(x, Wq, bq, Wk, bk, Wv, bv, Wo, bo):
    x = np.asarray(x, dtype=np.float32)
    Wq = np.asarray(Wq, dtype=np.float32)
    bq = np.asarray(bq, dtype=np.float32)
    Wk = np.asarray(Wk, dtype=np.float32)
    bk = np.asarray(bk, dtype=np.float32)
    Wv = np.asarray(Wv, dtype=np.float32)
    bv = np.asarray(bv, dtype=np.float32)
    Wo = np.asarray(Wo, dtype=np.float32)
    bo = np.asarray(bo, dtype=np.float32)

    if "nc" not in _cache:
        _cache["nc"] = _build()
    nc = _cache["nc"]

    negm16, id16 = _consts()
    scale = 1.0 / np.sqrt(np.float32(D_HEAD))
    in_maps = []
    for core in range(N_CORES):
        b, g = divmod(core, HPC)
        csl = slice(CSL * g, CSL * (g + 1))
        wo_loc = Wo[csl, :]  # [256, 1024]
        wo_pack = np.ascontiguousarray(
            wo_loc.reshape(2, 128, D_MODEL).transpose(1, 0, 2).reshape(
                128, 2 * D_MODEL))
        bqk = np.stack([
            bq[csl][0:128] * scale, bq[csl][128:256] * scale,
            bk[csl][0:128], bk[csl][128:256],
        ], axis=1).astype(np.float32)
        wqkv = np.concatenate([
            _pack_w(Wq[:, csl] * (scale * WS)),
            _pack_w(Wk[:, csl] * WS),
            _pack_w(Wv[:, csl] * WS),
            wo_pack * WS,
        ], axis=1)
        cst32 = np.concatenate([
            bqk,
            np.broadcast_to(WS * bv[None, csl], (128, CSL)),
            (bo / HPC).reshape(NT_D, 128).T,
            np.broadcast_to(bv[None, csl], (128, CSL)),
        ], axis=1).astype(np.float32)
        borow = np.zeros((128, 1024), dtype=np.float32)
        borow[0, :] = (WS * WS / HPC) * bo
        cst16 = np.concatenate([
            negm16.astype(np.float32), id16.astype(np.float32),
            _pack_w(Wv[:, csl]),
            wo_pack,
            _pack_w(np.ascontiguousarray(x[b].T[:, 0:128])),
            borow,
        ], axis=1).astype(np.float16)
        in_maps.append({
            "xt": _to8(np.ascontiguousarray(x[b].T)),
            "wqkv": _to8(wqkv),
            "cst32": np.ascontiguousarray(cst32),
            "cst16": np.ascontiguousarray(cst16),
        })

    # the axon terminal occasionally reports a transient
    # NRT_EXEC_UNIT_UNRECOVERABLE; retries with backoff recover it
    import time as _time
    for attempt in range(3):
        try:
            res = bass_utils.run_bass_kernel_spmd(
                nc, in_maps, core_ids=list(range(N_CORES)))
            break
        except Exception:
            if attempt == 2:
                raise
            _time.sleep(5.0 * (attempt + 1))

    full = np.empty((BATCH, SEQ, D_MODEL), dtype=np.float32)
    for core in range(N_CORES):
        b, g = divmod(core, HPC)
        o = np.asarray(res.results[core]["out"]).astype(np.float32)
        for sj in range(NQ):
            full[b, 512 * sj:512 * (sj + 1), 256 * g:256 * (g + 1)] = \
                o[256 * sj:256 * (sj + 1), :].T
    return full


# revision 45
# speedup vs baseline: 1.2216x; 1.0016x over previous
import sys

sys.path.insert(0, '/opt/trn_rl_repo')

import ml_dtypes
import numpy as np
import concourse.bass as bass
import concourse.mybir as mybir
import concourse.tile as tile
from concourse import bacc, bass_utils

F32 = mybir.dt.float32
BF16 = mybir.dt.bfloat16
FP16 = mybir.dt.float16
FP8 = mybir.dt.float8e4
AF = mybir.ActivationFunctionType
DR = mybir.MatmulPerfMode.DoubleRow
MUL = mybir.AluOpType.mult
ADD = mybir.AluOpType.add

D_MODEL = 1024
N_HEADS = 16
D_HEAD = 64
SEQ = 2048
BATCH = 2
N_CORES = 8
HPC = 4             # heads per core
CSL = HPC * D_HEAD  # 256: qkv feature slice per core
NT_D = D_MODEL // 128  # 8
NQ = SEQ // 512     # 4 q-chunks
GROUPS = [[0, 1, 2, 3], [4, 5, 6, 7]]
WS = 32.0           # fp8 weight pre-scale (host side)

_cache = {}


def _build():
    nc = bacc.Bacc("TRN2", target_bir_lowering=False, debug=False,
                   num_devices=N_CORES)
    xt_in = nc.dram_tensor("xt", [D_MODEL, SEQ], FP8, kind="ExternalInput").ap()
    w_in = nc.dram_tensor("wqkv", [128, 4 * NT_D * CSL], FP8,
                          kind="ExternalInput").ap()
    c32_in = nc.dram_tensor("cst32", [128, 4 + 2 * CSL + NT_D], F32,
                            kind="ExternalInput").ap()
    c16_in = nc.dram_tensor("cst16", [128, 384 + 2 * NT_D * CSL + 2048], FP16,
                            kind="ExternalInput").ap()
    out = nc.dram_tensor("out", [NQ * 256, 512], FP16,
                         kind="ExternalOutput").ap()

    with tile.TileContext(nc) as tc:
        _body(nc, tc, xt_in, w_in, c32_in, c16_in, out)
    nc.compile()
    return nc


def _body(nc, tc, xt_in, w_in, c32_in, c16_in, out):
    from contextlib import ExitStack
    ctx = ExitStack()
    with ctx:
        const = ctx.enter_context(tc.tile_pool(name="const", bufs=1))
        wpool = ctx.enter_context(tc.tile_pool(name="wpool", bufs=1))
        xtpool = ctx.enter_context(tc.tile_pool(name="xtpool", bufs=1))
        qkpool = ctx.enter_context(tc.tile_pool(name="qkpool", bufs=1))
        vpool = ctx.enter_context(tc.tile_pool(name="vpool", bufs=1))
        htpool = ctx.enter_context(tc.tile_pool(name="htpool", bufs=1))
        exp_pool = ctx.enter_context(tc.tile_pool(name="exp_pool", bufs=6))
        misc_c = ctx.enter_context(tc.tile_pool(name="misc_c", bufs=2))
        stpool = ctx.enter_context(tc.tile_pool(name="stpool", bufs=2))
        drpool = ctx.enter_context(tc.tile_pool(name="drpool", bufs=1, space="DRAM"))
        ps_pair = ctx.enter_context(tc.tile_pool(name="ps_pair", bufs=2, space="PSUM"))
        ps_att = ctx.enter_context(tc.tile_pool(name="ps_att", bufs=1, space="PSUM"))
        ps_m = ctx.enter_context(tc.tile_pool(name="ps_m", bufs=2, space="PSUM"))

        # ---- startup: few fat DMAs (HWDGE desc-gen is ~630ns each) ----
        xT = xtpool.tile([128, NT_D * SEQ], FP8, name="xT", tag="xT")
        xTv = xT.rearrange("p (d s) -> p d s", s=SEQ)
        xdr = xt_in.rearrange("(d p) s -> p d s", p=128)
        nc.sync.dma_start(xTv[:, 0:4, 0:512], xdr[:, 0:4, 0:512])
        wt = wpool.tile([128, 4 * NT_D * CSL], FP8, name="w_all")
        nc.scalar.dma_start(wt[:, 0:NT_D * CSL], w_in[:, 0:NT_D * CSL])
        nc.sync.dma_start(xTv[:, 4:8, 0:512], xdr[:, 4:8, 0:512])
        nc.scalar.dma_start(wt[:, NT_D * CSL:2 * NT_D * CSL],
                            w_in[:, NT_D * CSL:2 * NT_D * CSL])
        c16 = const.tile([128, 384 + 2 * NT_D * CSL + 2048], FP16)
        nc.scalar.dma_start(c16[:, 0:384], c16_in[:, 0:384])
        c32 = const.tile([128, 4 + 2 * CSL + NT_D], F32)
        nc.scalar.dma_start(c32[:], c32_in[:])
        nc.scalar.dma_start(wt[:, 2 * NT_D * CSL:], w_in[:, 2 * NT_D * CSL:])
        nc.sync.dma_start(xTv[:, :, 512:SEQ], xdr[:, :, 512:SEQ])
        nc.scalar.dma_start(c16[:, 384:], c16_in[:, 384:])
        wq_t = wt[:, 0:NT_D * CSL]
        wk_t = wt[:, NT_D * CSL:2 * NT_D * CSL]
        wv_t = wt[:, 2 * NT_D * CSL:3 * NT_D * CSL]
        wo_t = wt[:, 3 * NT_D * CSL:4 * NT_D * CSL]
        bqk = c32[:, 0:4]
        bv32 = c32[:, 4:4 + CSL]
        boT4 = c32[:, 4 + CSL:4 + CSL + NT_D]
        negm16 = c16[:, 0:256]
        id16 = c16[:, 256:384]
        wv16 = c16[:, 384:384 + NT_D * CSL]
        wv16v = wv16.rearrange("p (d m) -> p d m", m=CSL)
        wo16v = c16[:, 384 + NT_D * CSL:384 + 2 * NT_D * CSL].rearrange(
            "p (s c) -> p s c", c=D_MODEL)
        xT16v = c16[:, 384 + 2 * NT_D * CSL:].rearrange(
            "p (d s) -> p d s", s=128)
        bv16 = c32[:, 4 + CSL + NT_D:4 + 2 * CSL + NT_D]
        ones16 = const.tile([1, 128], FP16)
        nc.vector.memset(ones16[:], 1.0)
        actwarm = const.tile([1, 4], FP16)
        nc.vector.memset(actwarm[:], 0.0)
        nc.scalar.activation(actwarm[:], actwarm[:], AF.Exp)
        onesw = const.tile([1, 512], FP16)
        nc.vector.memset(onesw[:], 1.0)
        bo16 = c16[0:1, 384 + 2 * NT_D * CSL + 1024:]

        # ---- persistent activations ----
        qt, kt = [], []
        for ci in range(2):
            qt.append(qkpool.tile([128, SEQ], FP16, name=f"qt{ci}", tag=f"qt{ci}"))
            kt.append(qkpool.tile([128, SEQ], FP16, name=f"kt{ci}", tag=f"kt{ci}"))
        hT = htpool.tile([128, 2 * SEQ], FP8, name="hT", tag="hT")
        hTv = hT.rearrange("p (s c) -> p s c", c=SEQ)
        vtp = [vpool.tile([128, 2 * HPC * 68], FP8, name=f"vt{j}", tag=f"vt{j}")
               for j in range(NT_D)]
        wav = wt.rearrange("p (w d m) -> p w d m", w=4, m=CSL)
        wqv = wav[:, 0]
        wkv = wav[:, 1]
        wvv = wav[:, 2]
        wov = wt.rearrange("p (w s c) -> p w s c", w=4, c=D_MODEL)[:, 3]

        def emit_qk_half(sj, ci, wv_, bcol, dst, half, state):
            """Half a q/k projection slice: 2 DR matmuls (+ bias on 2nd)."""
            if half == 0:
                state["pp"] = ps_m.tile([128, 512], F32, name="pp", tag="m")
            pp = state["pp"]
            for dj in (4 * half, 4 * half + 2):
                nc.tensor.matmul(
                    pp[:],
                    wv_[:, dj:dj + 2, 128 * ci:128 * (ci + 1)],
                    xTv[:, dj:dj + 2, 512 * sj:512 * (sj + 1)],
                    start=(dj == 0), stop=(dj == 6),
                    perf_mode=DR, skip_group_check=True)
            if half == 1:
                nc.vector.tensor_scalar(
                    dst[ci][:, 512 * sj:512 * (sj + 1)], pp[:],
                    1.0 / WS, bqk[:, bcol + ci:bcol + ci + 1],
                    op0=MUL, op1=ADD)

        def emit_qk(sj, ci, wv_, bcol, dst):
            state = {}
            emit_qk_half(sj, ci, wv_, bcol, dst, 0, state)
            emit_qk_half(sj, ci, wv_, bcol, dst, 1, state)

        def emit_v_half(sj, sl, half, state):
            """Half a v s-tile: 2 DR matmuls (+ ones memset and add on 2nd)."""
            si = 4 * sj + sl
            if half == 0:
                state["pv"] = ps_m.tile([128, 512], F32, name="pv", tag="m")
            pv = state["pv"]
            for dj in (4 * half, 4 * half + 2):
                nc.tensor.matmul(
                    pv[:, 0:CSL],
                    xTv[:, dj:dj + 2, 128 * si:128 * (si + 1)],
                    wvv[:, dj:dj + 2, :],
                    start=(dj == 0), stop=(dj == 6),
                    perf_mode=DR, skip_group_check=True)
            if half == 1:
                j, slot = divmod(si, 2)
                vv = vtp[j].rearrange("p (s h e) -> p s h e", s=2, e=68)
                nc.vector.memset(vv[:, slot, :, 64:65], 1.0)
                nc.vector.tensor_add(
                    vv[:, slot, :, 0:64],
                    pv[:, 0:CSL].rearrange("p (h e) -> p h e", e=64),
                    bv32.rearrange("p (h e) -> p h e", e=64))

        def emit_v(sj, sl):
            state = {}
            emit_v_half(sj, sl, 0, state)
            emit_v_half(sj, sl, 1, state)

        pending = []
        pending_wo = []

        def pop_pending(n):
            for _ in range(min(n, len(pending))):
                pending.pop(0)()

        def pop_wo(n=100):
            for _ in range(min(n, len(pending_wo))):
                pending_wo.pop(0)()

        deferred = [None]

        def flush_norm():
            if deferred[0] is not None:
                f = deferred[0]
                deferred[0] = None
                f()

        def emit_norm(sj, ci, pa):
            # softmax denominators -> reciprocal -> broadcast -> scale
            rt = misc_c.tile([1, 1024], FP16, name="rt", tag="rt")
            with nc.allow_low_precision(reason="softmax 1/denom in fp16"):
                nc.vector.reciprocal(rt[:, 0:512], pa[0][64:65, :])
                nc.vector.reciprocal(rt[:, 512:1024], pa[1][64:65, :])
            pb = ps_m.tile([128, 512], F32, name="pb", tag="m")
            nc.tensor.matmul(pb[0:64, :], ones16[0:1, 0:64], rt[:, 0:512],
                             start=True, stop=True)
            nc.tensor.matmul(pb[64:128, :], ones16[0:1, 0:64], rt[:, 512:1024],
                             start=True, stop=True, tile_position=(0, 64))
            bc = misc_c.tile([128, 512], FP16, name="bc", tag="bc")
            nc.vector.tensor_scalar_add(bc[:], pb[:], 0.0)
            pop_pending(2)  # cover the bc wait with filler matmuls
            for hh in range(2):
                nc.vector.tensor_mul(
                    hTv[64 * hh:64 * (hh + 1), ci, 512 * sj:512 * (sj + 1)],
                    pa[hh][0:64, :], bc[64 * hh:64 * (hh + 1), :])

        def emit_attention(sj):
            npairs = 2 * sj + 2
            # diag pairs first (fast psum refill), then off-diag ascending
            order = [2 * sj, 2 * sj + 1] + list(range(2 * sj))
            for ci in range(2):
                pa = [ps_att.tile([65, 512], F32, name=f"pa{hh}", tag=f"att{hh}")
                      for hh in range(2)]

                def emit_av(j, etv, pos):
                    """attn@V for pair j (software-pipelined one pair behind)."""
                    is_diag_hi = (j == 2 * sj + 1)
                    is_diag_lo = (j == 2 * sj)
                    last = (pos == npairs - 1)
                    for hh in range(2):
                        h_local = 2 * ci + hh
                        vv = vtp[j].rearrange(
                            "p (s h e) -> p s h e", s=2, e=68)[:, :, h_local, 0:65]
                        if is_diag_hi:
                            # tiles r2 (c0=256), r3 (c0=384)
                            nc.tensor.matmul(
                                pa[hh][:, 384:512], vv, etv[:, hh, :, 384:512],
                                start=False, stop=False,
                                perf_mode=DR, skip_group_check=True)
                            nc.tensor.matmul(
                                pa[hh][:, 256:384], vv[:, 0, :],
                                etv[:, hh, 0, 256:384],
                                start=False, stop=last, skip_group_check=True)
                        elif is_diag_lo:
                            # tiles r0 (c0=0), r1 (c0=128)
                            nc.tensor.matmul(
                                pa[hh][:, 128:512], vv, etv[:, hh, :, 128:512],
                                start=(pos == 0), stop=False,
                                perf_mode=DR, skip_group_check=True)
                            nc.tensor.matmul(
                                pa[hh][:, 0:128], vv[:, 0, :],
                                etv[:, hh, 0, 0:128],
                                start=False, stop=last, skip_group_check=True)
                        else:
                            nc.tensor.matmul(
                                pa[hh][:, 0:512], vv, etv[:, hh, :, 0:512],
                                start=False, stop=last,
                                perf_mode=DR, skip_group_check=True)

                prev = None
                prev2 = None
                prev3 = None
                for pos, j in enumerate(order):
                    et = exp_pool.tile([128, 2048], FP8, name="et", tag="et")
                    etv = et.rearrange("p (h s c) -> p h s c", h=2, c=512)
                    for slot in range(2):
                        ki = 2 * j + slot
                        r = ki - 4 * sj
                        c0 = 0 if r < 0 else 128 * r
                        ps = ps_pair.tile([128, 1024], F32, name="ps", tag="sp")
                        psv = ps.rearrange("p (h c) -> p h c", h=2)
                        for hh in range(2):
                            p0 = 64 * hh
                            nc.tensor.matmul(
                                ps[:, 512 * hh + c0:512 * (hh + 1)],
                                kt[ci][p0:p0 + 64, 128 * ki:128 * (ki + 1)],
                                qt[ci][p0:p0 + 64, 512 * sj + c0:512 * (sj + 1)],
                                start=True, stop=(r < 0))
                        if r >= 0:
                            # causal mask via PE, both heads
                            nc.tensor.matmul(
                                psv[:, :, c0:c0 + 128],
                                id16[:], negm16[:],
                                start=False, stop=True, skip_group_check=True)
                        nc.scalar.activation(
                            etv[:, :, slot, c0:512],
                            psv[:, :, c0:512],
                            AF.Exp)
                    if pos == 0:
                        flush_norm()
                        pop_wo(2)
                        if sj == 0 and ci == 0:
                            pop_pending(4)
                    if prev3 is not None:
                        emit_av(*prev3)
                        pop_wo(2)
                        pop_pending(1)
                    prev3 = prev2
                    prev2 = prev
                    prev = (j, etv, pos)
                if sj == 0 and ci == 0:
                    pop_pending(8)
                for pv in (prev3, prev2, prev):
                    if pv is not None:
                        emit_av(*pv)
                deferred[0] = lambda s=sj, c=ci, p=pa: emit_norm(s, c, p)
                pop_pending(1)

        wo_stage = {}
        wo_done = set()
        out_dmas = []

        def emit_wo_t(sj, t):
            """One transposed out block [128 dout, 512 seq] = Wo_t.T @ hT."""
            if sj not in wo_stage:
                wo_stage[sj] = (
                    [stpool.tile([128, 2 * 512], FP16, name=f"pst{sj}_{q}",
                                 tag=f"pst{q}") for q in range(4)],
                    drpool.tile([D_MODEL, 512], FP16, name=f"prs{sj}",
                                tag=f"prs{sj}"))
            pstage, prs = wo_stage[sj]
            pstage = pstage[t // 2]
            po = ps_m.tile([128, 512], F32, name="po", tag="m")
            tail = (sj == NQ - 1)
            nc.tensor.matmul(
                po[:],
                wov[:, :, 128 * t:128 * (t + 1)],
                hTv[:, :, 512 * sj:512 * (sj + 1)],
                start=True, stop=not tail, perf_mode=DR, skip_group_check=True)
            c0w = 128 if sj == 0 else 0
            dst = pstage.rearrange("p (t c) -> p t c", c=512)[:, t % 2, c0w:512]
            if tail:
                # bias via PE; staging split ACT/DVE (ACT idles at the tail)
                nc.tensor.matmul(
                    po[:], bo16[0:1, 128 * t:128 * (t + 1)], onesw[0:1, :],
                    start=False, stop=True, skip_group_check=True)
                if t % 2 == 0:
                    nc.scalar.mul(dst, po[:, c0w:512], 1.0 / (WS * WS))
                else:
                    nc.vector.tensor_scalar(
                        dst, po[:, c0w:512], 1.0 / (WS * WS), None, op0=MUL)
            else:
                nc.vector.tensor_scalar(
                    dst, po[:, c0w:512],
                    1.0 / (WS * WS), boT4[:, t:t + 1], op0=MUL, op1=ADD)
            if t % 2 == 1:
                eng = nc.scalar if sj == NQ - 1 else nc.sync
                eng.dma_start(
                    prs.rearrange("(t p) c -> p t c", p=128)[:, t - 1:t + 1, :],
                    pstage.rearrange("p (t c) -> p t c", c=512)[:, :, :])
            if t == NT_D - 1:
                pro = drpool.tile([256, 512], FP16, name=f"pro{sj}",
                                  tag=f"pro{sj}")
                nc.gpsimd.collective_compute(
                    "ReduceScatter", mybir.AluOpType.add,
                    replica_groups=GROUPS, ins=[prs[:]], outs=[pro[:]])
                out_dmas.append(
                    lambda s=sj, p=pro: nc.sync.dma_start(
                        out[256 * s:256 * (s + 1), :], p[:]))

        def emit_side():
            """fp16 recompute of out rows 0..127: little attention averaging
            there, so fp8 value-chain noise is too big for those rows."""
            if 0 not in wo_stage:
                wo_stage[0] = (
                    [stpool.tile([128, 2 * 512], FP16, name=f"pst0_{q}",
                                 tag=f"pst{q}") for q in range(4)],
                    drpool.tile([D_MODEL, 512], FP16, name="prs0",
                                tag="prs0"))
            pstage_q, prs = wo_stage[0]
            pv = ps_m.tile([128, 512], F32, name="pv16", tag="m")
            for dj in range(NT_D):
                nc.tensor.matmul(
                    pv[:, 0:CSL], xT16v[:, dj, :], wv16v[:, dj, :],
                    start=(dj == 0), stop=(dj == NT_D - 1),
                    skip_group_check=True)
            vt16 = vpool.tile([128, HPC * 65], FP16, name="vt16", tag="vt16")
            v16v = vt16.rearrange("p (h e) -> p h e", e=65)
            nc.vector.memset(v16v[:, :, 64:65], 1.0)
            nc.vector.tensor_add(
                v16v[:, :, 0:64],
                pv[:, 0:CSL].rearrange("p (h e) -> p h e", e=64),
                bv16.rearrange("p (h e) -> p h e", e=64))
            pa16 = ps_att.tile([65, 512], F32, name="pa16", tag="att0")
            for ci in range(2):
                ps16 = ps_pair.tile([128, 1024], F32, name="ps16", tag="sp")
                psv16 = ps16.rearrange("p (h c) -> p h c", h=2)
                for hh in range(2):
                    p0 = 64 * hh
                    nc.tensor.matmul(
                        ps16[:, 512 * hh:512 * hh + 128],
                        kt[ci][p0:p0 + 64, 0:128], qt[ci][p0:p0 + 64, 0:128],
                        start=True, stop=False)
                nc.tensor.matmul(
                    psv16[:, :, 0:128],
                    id16[:], negm16[:],
                    start=False, stop=True, skip_group_check=True)
                et16 = misc_c.tile([128, 256], FP16, name="et16", tag="et16")
                nc.scalar.activation(
                    et16.rearrange("p (h c) -> p h c", c=128)[:, :, :],
                    psv16[:, :, 0:128], AF.Exp)
                for hh in range(2):
                    hl = 2 * ci + hh
                    nc.tensor.matmul(
                        pa16[:, 128 * hl:128 * (hl + 1)],
                        vt16[:, 65 * hl:65 * hl + 65],
                        et16[:, 128 * hh:128 * (hh + 1)],
                        start=(hl == 0), stop=(hl == 3),
                        skip_group_check=True)
            rt16 = misc_c.tile([1, 512], FP16, name="rt16", tag="rt16")
            with nc.allow_low_precision(reason="softmax 1/denom in fp16"):
                nc.vector.reciprocal(rt16[:], pa16[64:65, 0:512])
            pb16 = ps_m.tile([128, 512], F32, name="pb16", tag="m")
            nc.tensor.matmul(pb16[0:64, :], ones16[0:1, 0:64], rt16[:],
                             start=True, stop=True)
            bc16 = misc_c.tile([128, 512], FP16, name="bc16", tag="bc16")
            nc.vector.tensor_scalar_add(bc16[0:64, :], pb16[0:64, :], 0.0)
            h16 = htpool.tile([128, 2 * 128], FP16, name="h16", tag="h16")
            h16v = h16.rearrange("p (s c) -> p s c", c=128)
            for ci in range(2):
                for hh in range(2):
                    hl = 2 * ci + hh
                    nc.vector.tensor_mul(
                        h16v[64 * hh:64 * (hh + 1), ci, :],
                        pa16[0:64, 128 * hl:128 * (hl + 1)],
                        bc16[0:64, 128 * hl:128 * (hl + 1)])
            for t in range(NT_D):
                po = ps_m.tile([128, 512], F32, name="po16", tag="m")
                for s in range(2):
                    nc.tensor.matmul(
                        po[:, 0:128],
                        wo16v[:, s, 128 * t:128 * (t + 1)],
                        h16v[:, s, :],
                        start=(s == 0), stop=(s == 1), skip_group_check=True)
                nc.vector.tensor_scalar(
                    pstage_q[t // 2].rearrange(
                        "p (t c) -> p t c", c=512)[:, t % 2, 0:128],
                    po[:, 0:128],
                    1.0, boT4[:, t:t + 1], op0=MUL, op1=ADD)

        # ---- pipeline over q-chunks ----
        # chunk-0 q/k run dj-major so PE consumes x slices as they arrive
        qk0 = [(wqv, 0, qt, 0), (wkv, 2, kt, 0),
               (wqv, 0, qt, 1), (wkv, 2, kt, 1)]
        pps0 = []
        for i in range(2):
            t = ps_pair.tile([128, 1024], F32, name=f"pp0_{i}", tag="sp")
            pps0 += [t[:, 0:512], t[:, 512:1024]]
        # ci0's q/k eagerly (attention ci0 starts as soon as they land);
        # everything else chunk-0 interleaves into attention(0) as fillers
        for dj in range(0, NT_D, 2):
            for u in (0, 1):
                wv_, bcol, dst, ci = qk0[u]
                nc.tensor.matmul(
                    pps0[u],
                    wv_[:, dj:dj + 2, 128 * ci:128 * (ci + 1)],
                    xTv[:, dj:dj + 2, 0:512],
                    start=(dj == 0), stop=(dj == 6),
                    perf_mode=DR, skip_group_check=True)
        for u in (0, 1):
            wv_, bcol, dst, ci = qk0[u]
            nc.vector.tensor_scalar(
                dst[ci][:, 0:512], pps0[u],
                1.0 / WS, bqk[:, bcol + ci:bcol + ci + 1],
                op0=MUL, op1=ADD)
        for sl in (2, 3):
            st = {}
            for half in range(2):
                pending.append(
                    lambda l=sl, h=half, t=st: emit_v_half(0, l, h, t))
        for u in (2, 3):
            wv_, bcol, dst, ci = qk0[u]
            st = {}

            def qk0_half(h, u=u, st=st):
                wv_, bcol, dst, ci = qk0[u]
                if h == 0:
                    st["pp"] = pps0[u]
                for dj in ((0, 2) if h == 0 else (4, 6)):
                    nc.tensor.matmul(
                        pps0[u],
                        wv_[:, dj:dj + 2, 128 * ci:128 * (ci + 1)],
                        xTv[:, dj:dj + 2, 0:512],
                        start=(dj == 0), stop=(dj == 6),
                        perf_mode=DR, skip_group_check=True)
                if h == 1:
                    nc.vector.tensor_scalar(
                        dst[ci][:, 0:512], pps0[u],
                        1.0 / WS, bqk[:, bcol + ci:bcol + ci + 1],
                        op0=MUL, op1=ADD)
            for half in range(2):
                pending.append(lambda h=half, f=qk0_half: f(h))
        for sl in (0, 1):
            st = {}
            for half in range(2):
                pending.append(
                    lambda l=sl, h=half, t=st: emit_v_half(0, l, h, t))
        for sj in range(NQ):
            if sj > 0 and sj - 1 not in wo_done:
                for u in range(NT_D):
                    pending_wo.append(lambda s=sj - 1, t=u: emit_wo_t(s, t))
            if sj + 1 < NQ:
                nsj = sj + 1
                for ci in range(2):
                    for wv_, bcol, dst in [(wqv, 0, qt), (wkv, 2, kt)]:
                        st = {}
                        for half in range(2):
                            pending.append(
                                lambda c=ci, s=nsj, w=wv_, b=bcol, d=dst,
                                h=half, t=st: emit_qk_half(s, c, w, b, d, h, t))
                for sl in range(4):
                    st = {}
                    for half in range(2):
                        pending.append(
                            lambda s=nsj, l=sl, h=half, t=st:
                            emit_v_half(s, l, h, t))
            emit_attention(sj)
            pop_pending(len(pending))
            if sj == 0:
                flush_norm()
                emit_side()
            if sj == NQ - 2:
                # inline the chunk-2 norm+wo so RS2 starts early enough that
                # the collective chain is off the critical path
                flush_norm()
                for t in range(NT_D):
                    emit_wo_t(sj, t)
                wo_done.add(sj)
        flush_norm()
        for t in range(NT_D):
            emit_wo_t(NQ - 1, t)
        for f in out_dmas:
            f()


def _consts():
    kk = np.arange(128)[:, None]
    qq = np.arange(128)[None, :]
    negm16 = np.where(kk <= qq, 0.0, -60000.0).astype(np.float16)
    negm16 = np.concatenate([negm16, negm16], axis=1)
    id16 = np.eye(128, dtype=np.float16)
    return negm16, id16


F8NP = ml_dtypes.float8_e4m3


def _to8(a):
    return np.clip(a, -224.0, 224.0).astype(F8NP)


def _pack_w(w):
    """[1024, C] -> [128, 8*C] with d-tile t at column block t."""
    c = w.shape[1]
    return np.ascontiguousarray(
        w.reshape(NT_D, 128, c).transpose(1, 0, 2).reshape(128, NT_D * c))


def kernel(x, Wq, bq, Wk, bk, Wv, bv, Wo, bo):
    x = np.asarray(x, dtype=np.float32)
    Wq = np.asarray(Wq, dtype=np.float32)
    bq = np.asarray(bq, dtype=np.float32)
    Wk = np.asarray(Wk, dtype=np.float32)
    bk = np.asarray(bk, dtype=np.float32)
    Wv = np.asarray(Wv, dtype=np.float32)
    bv = np.asarray(bv, dtype=np.float32)
    Wo = np.asarray(Wo, dtype=np.float32)
    bo = np.asarray(bo, dtype=np.float32)

    if "nc" not in _cache:
        _cache["nc"] = _build()
    nc = _cache["nc"]

    negm16, id16 = _consts()
    scale = 1.0 / np.sqrt(np.float32(D_HEAD))
    in_maps = []
    for core in range(N_CORES):
        b, g = divmod(core, HPC)
        csl = slice(CSL * g, CSL * (g + 1))
        wo_loc = Wo[csl, :]  # [256, 1024]
        wo_pack = np.ascontiguousarray(
            wo_loc.reshape(2, 128, D_MODEL).transpose(1, 0, 2).reshape(
                128, 2 * D_MODEL))
        bqk = np.stack([
            bq[csl][0:128] * scale, bq[csl][128:256] * scale,
            bk[csl][0:128], bk[csl][128:256],
        ], axis=1).astype(np.float32)
        wqkv = np.concatenate([
            _pack_w(Wq[:, csl] * (scale * WS)),
            _pack_w(Wk[:, csl] * WS),
            _pack_w(Wv[:, csl] * WS),
            wo_pack * WS,
        ], axis=1)
        cst32 = np.concatenate([
            bqk,
            np.broadcast_to(WS * bv[None, csl], (128, CSL)),
            (bo / HPC).reshape(NT_D, 128).T,
            np.broadcast_to(bv[None, csl], (128, CSL)),
        ], axis=1).astype(np.float32)
        borow = np.zeros((128, 1024), dtype=np.float32)
        borow[0, :] = (WS * WS / HPC) * bo
        cst16 = np.concatenate([
            negm16.astype(np.float32), id16.astype(np.float32),
            _pack_w(Wv[:, csl]),
            wo_pack,
            _pack_w(np.ascontiguousarray(x[b].T[:, 0:128])),
            borow,
        ], axis=1).astype(np.float16)
        in_maps.append({
            "xt": _to8(np.ascontiguousarray(x[b].T)),
            "wqkv": _to8(wqkv),
            "cst32": np.ascontiguousarray(cst32),
            "cst16": np.ascontiguousarray(cst16),
        })

    # the axon terminal occasionally reports a transient
    # NRT_EXEC_UNIT_UNRECOVERABLE; retries with backoff recover it
    import time as _time
    for attempt in range(3):
        try:
            res = bass_utils.run_bass_kernel_spmd(
                nc, in_maps, core_ids=list(range(N_CORES)))
            break
        except Exception:
            if attempt == 2:
                raise
            _time.sleep(5.0 * (attempt + 1))

    full = np.empty((BATCH, SEQ, D_MODEL), dtype=np.float32)
    for core in range(N_CORES):
        b, g = divmod(core, HPC)
        o = np.asarray(res.results[core]["out"]).astype(np.float32)
        for sj in range(NQ):
            full[b, 512 * sj:512 * (sj + 1), 256 * g:256 * (g + 1)] = \
                o[256 * sj:256 * (sj + 1), :].T
    return full
